# revision 51
# baseline (speedup 1.0000x reference)
"""EvolvedAttention Trainium2 Bass kernel (v2).

Full inputs -> full output. Sharding: 8 cores = 2 batches x 4 query-row
slices. Each core computes K/V/attention for its (batch, row-slice) with
all 16 heads; host slices inputs and concatenates row-slice outputs.

v2 design (from ntff trace of v1: DVE 73% busy on top-k counting, PE 38%
and cold):
  - Q/K/V projections in fp8e4 + DoubleRow (weights x32 host-side, folded
    back via Wo/32; cosine normalization cancels the scale for q/k).
  - gate/temp/Wo in fp16.
  - KnT (head-major [65, S], ones row for the threshold trick) and the
    gate stay SBUF-resident; no DRAM staging.
  - top-k threshold found on 4x-subsampled keys (strided matmul rhs),
    3 count-iterations split across ACT (Sign+accum), GPSIMD and DVE,
    bracketed false-position in "acc" space (acc = #ge - #lt).
  - scores recomputed transposed with threshold folded in (K=65), exp on
    ACT PSUM->fp8, mask on GPSIMD (em8 = [z>=0]*e8), AV in fp8 DoubleRow
    with a ones column in V8 for the softmax denominator.
  - selection of group g pipelines against attention of group g-1; the
    V projection fills the group-0 selection bubble.
"""

import os
import numpy as np

import concourse.bass as bass
import concourse.mybir as mybir
import concourse.tile as tile
from concourse import bacc

FP32 = mybir.dt.float32
FP16 = mybir.dt.float16
FP8 = mybir.dt.float8e4
U8 = mybir.dt.uint8
AF = mybir.ActivationFunctionType
ALU = mybir.AluOpType
DR = mybir.MatmulPerfMode.DoubleRow

WSCALE = 32.0


class Cfg:
    def __init__(self):
        self.S = 2048
        self.D = 1024
        self.NH = 16
        self.DH = 64
        self.RS = 512
        self.KK = self.S // 4          # top-k
        self.SUB = 4                   # key subsample for threshold search
        self.SS = self.S // self.SUB   # sampled keys (512)
        self.DCH = self.D // 128       # 8
        self.KC = self.S // 128        # 16
        self.RC = self.RS // 128       # 4
        self.HP = self.NH // 2         # 8
        self.GROUP = 4
        self.NG = self.NH // self.GROUP
        self.n_sel_iters = 2
        # target in acc space: acc = 2*c - SS, c target = KK/SUB
        self.ATGT = float(2 * (self.KK // self.SUB) - self.SS)  # -256
        self.slope0 = 2.0 * 2.8 * self.SS  # d(acc)/dt estimate


def build(cfg: Cfg, with_bias: bool):
    nc = bacc.Bacc()
    S, D, NH, DH, RS = cfg.S, cfg.D, cfg.NH, cfg.DH, cfg.RS
    DCH, KC, RC, HP = cfg.DCH, cfg.KC, cfg.RC, cfg.HP
    SS, G, NG = cfg.SS, cfg.GROUP, cfg.NG

    x8T = nc.dram_tensor("x8T", [128, DCH, S], FP8, kind="ExternalInput")
    x16T = nc.dram_tensor("x16T", [128, DCH, RS], FP16, kind="ExternalInput")
    xs = nc.dram_tensor("xs", [128, RC, D], FP32, kind="ExternalInput")
    Wq = nc.dram_tensor("Wq", [128, DCH, D], FP8, kind="ExternalInput")
    Wk = nc.dram_tensor("Wk", [128, DCH, D], FP8, kind="ExternalInput")
    Wv = nc.dram_tensor("Wv", [128, DCH, D], FP8, kind="ExternalInput")
    Wg = nc.dram_tensor("Wg", [128, DCH, D], FP16, kind="ExternalInput")
    Wo = nc.dram_tensor("Wo", [128, HP, D], FP16, kind="ExternalInput")
    Wt = nc.dram_tensor("Wt", [128, DCH], FP8, kind="ExternalInput")
    bt = nc.dram_tensor("bt", [1, 1], FP32, kind="ExternalInput")
    if with_bias:
        bq = nc.dram_tensor("bq", [1, D], FP16, kind="ExternalInput")
        bk = nc.dram_tensor("bk", [1, D], FP16, kind="ExternalInput")
        bv = nc.dram_tensor("bv", [1, D], FP16, kind="ExternalInput")
        bg = nc.dram_tensor("bg", [1, D], FP16, kind="ExternalInput")
        bo = nc.dram_tensor("bo", [1, D], FP16, kind="ExternalInput")
    out = nc.dram_tensor("out", [128, RC, D], FP32, kind="ExternalOutput")

    with tile.TileContext(nc) as tc:
        with (
            tc.tile_pool(name="persist", bufs=1) as pp,
            tc.tile_pool(name="psum", bufs=2, space="PSUM") as ps,
        ):
            # ---------------- persistent tiles (phase A) ----------------
            ident = pp.tile([128, 128], FP16, tag="ident")
            from concourse.masks import make_identity
            make_identity(nc, ident[:])
            ones_h = pp.tile([1, 128], FP16, tag="ones_h")
            nc.vector.memset(ones_h[:], 1.0)
            KnT = [pp.tile([65, S], FP16, tag=f"knt{h}", name=f"knt{h}")
                   for h in range(NH)]
            QnT = [pp.tile([65, RS], FP16, tag=f"qnt{h}", name=f"qnt{h}")
                   for h in range(NH)]
            for h in range(NH):
                nc.gpsimd.memset(KnT[h][64:65, :], 1.0)
            gate16 = pp.tile([128, RC, D], FP16, tag="gate16")
            invt128 = pp.tile([128, 1], FP32, tag="invt128")
            bt_t = pp.tile([1, 1], FP32, tag="bt")
            nc.sync.dma_start(bt_t[:], bt[:])
            wt_t = pp.tile([128, DCH], FP8, tag="wt")
            nc.sync.dma_start(wt_t[:], Wt[:])
            bias_t = {}
            if with_bias:
                for nm, dram in (("bq", bq), ("bk", bk), ("bv", bv),
                                 ("bg", bg), ("bo", bo)):
                    t = pp.tile([1, D], FP16, tag=nm, name=f"b_{nm}")
                    nc.sync.dma_start(t[:], dram[:])
                    bias_t[nm] = t

            def pt1024(name):
                """projection psum: [128,1024] (2 banks), ring of 2."""
                return ps.tile([128, 1024], FP32, tag="pt", bufs=2,
                               padded_shape=[128, 1024], name=name)

            def ps512(name, shape=None, dtype=FP32):
                """small psum ring (transposes, sel-scores, gate, temp)."""
                return ps.tile(shape or [128, 512], dtype, tag="tps",
                               bufs=2, padded_shape=[128, 512], name=name)

            # ---------------- helpers ----------------
            def proj_fp8(xt8, w_dram, bias_row, n_chunks, wpool, wtag):
                """fp8 DoubleRow projection; yields (j, pt) with pt a
                [128,1024] psum row-chunk."""
                w = wpool.tile([128, DCH, D], FP8, tag=wtag, name=wtag,
                               bufs=2)
                nc.sync.dma_start(w[:], w_dram[:])
                for j in range(n_chunks):
                    pt = pt1024(f"pt_{wtag}")
                    for n in range(2):
                        sl = slice(n * 512, (n + 1) * 512)
                        for cp in range(DCH // 2):
                            nc.tensor.matmul(
                                pt[:, sl],
                                xt8[:, 2 * cp : 2 * cp + 2,
                                    j * 128 : (j + 1) * 128],
                                w[:, 2 * cp : 2 * cp + 2, sl],
                                start=(cp == 0),
                                stop=(cp == DCH // 2 - 1 and bias_row is None),
                                perf_mode=DR)
                        if bias_row is not None:
                            nc.tensor.matmul(
                                pt[:, sl], ones_h[:], bias_row[:, sl],
                                start=False, stop=True)
                    yield j, pt

            def proj_fp16_half(xt16, w_dram, bias_row, n, n_chunks, wpool,
                               wtag):
                """one 512-wide output half of an fp16 projection; yields
                (j, pt) psum tiles [128,512]."""
                w = wpool.tile([128, DCH, 512], FP16, tag=wtag, name=wtag,
                               bufs=2)
                nc.sync.dma_start(w[:], w_dram[:, :, n * 512 : (n + 1) * 512])
                for j in range(n_chunks):
                    pt = ps512(f"pt_{wtag}{n}")
                    for c in range(DCH):
                        nc.tensor.matmul(
                            pt[:],
                            xt16[:, c, j * 128 : (j + 1) * 128],
                            w[:, c, :],
                            start=(c == 0),
                            stop=(c == DCH - 1 and bias_row is None))
                    if bias_row is not None:
                        nc.tensor.matmul(
                            pt[:], ones_h[:],
                            bias_row[:, n * 512 : (n + 1) * 512],
                            start=False, stop=True)
                    yield j, pt

            def normalize_pair(sp, pt, dst16, extra_scale_ap):
                """cosine-normalize a [128,1024] psum row-chunk into
                dst16 [128, D] fp16."""
                sq = sp.tile([128, D], FP16, tag="sq", name="sq", bufs=3)
                nc.scalar.activation(sq[:], pt[:], AF.Square)
                n2 = sp.tile([128, NH], FP32, tag="n2", name="n2", bufs=3)
                nc.vector.tensor_reduce(
                    n2[:], sq[:].rearrange("p (h d) -> p h d", h=NH),
                    axis=mybir.AxisListType.X, op=ALU.add)
                rec = sp.tile([128, NH], FP32, tag="rec", name="rec", bufs=3)
                nc.vector.tensor_scalar_max(rec[:], n2[:], 1e-12)
                nc.vector.reciprocal(rec[:], rec[:])
                rsq = sp.tile([128, NH], FP32, tag="rsq", name="rsq", bufs=3)
                nc.scalar.activation(rsq[:], rec[:], AF.Sqrt)
                if extra_scale_ap is not None:
                    nc.vector.tensor_scalar(
                        out=rsq[:], in0=rsq[:], scalar1=extra_scale_ap,
                        scalar2=None, op0=ALU.mult)
                nc.vector.tensor_tensor(
                    dst16[:].rearrange("p (h d) -> p h d", h=NH),
                    pt[:].rearrange("p (h d) -> p h d", h=NH),
                    rsq[:].rearrange("p (h o) -> p h o", o=1)
                        .to_broadcast([128, NH, DH]),
                    ALU.mult)

            def transpose_to_heads(dst_of_head, src16, j, who):
                """src16 [128 rows, 1024] -> per-head [64, 128] blocks into
                dst_of_head(h)[0:64, j*128:(j+1)*128]."""
                for p in range(HP):
                    tps = ps.tile([128, 128], FP16, tag="tps", bufs=2,
                                  padded_shape=[128, 512], name=f"tps_{who}")
                    nc.tensor.transpose(
                        tps[:], src16[:, p * 128 : (p + 1) * 128], ident[:])
                    for hh in range(2):
                        h = 2 * p + hh
                        dst = dst_of_head(h)[0:64, j * 128 : (j + 1) * 128]
                        src = tps[hh * 64 : hh * 64 + 64, :]
                        if (p + hh + j) % 2 == 0:
                            nc.scalar.activation(dst, src, AF.Copy)
                        else:
                            nc.vector.tensor_copy(dst, src)

            # ================ phase A ================
            with tc.tile_pool(name="poolX8", bufs=1) as px:
                xt8 = px.tile([128, DCH, S], FP8, tag="xt8")
                nc.sync.dma_start(xt8[:], x8T[:])

                with (
                    tc.tile_pool(name="poolA", bufs=1) as pa,
                    tc.tile_pool(name="wpoolA", bufs=2) as wpa,
                ):
                    xt16 = pa.tile([128, DCH, RS], FP16, tag="xt16")
                    nc.sync.dma_start(xt16[:], x16T[:])

                    # --- K projection -> KnT (resident) ---
                    for j, pt in proj_fp8(xt8, Wk, bias_t.get("bk"),
                                          KC, wpa, "w8"):
                        kn = pa.tile([128, D], FP16, tag="kn", name="kn",
                                     bufs=3)
                        normalize_pair(pa, pt, kn, None)
                        transpose_to_heads(lambda h: KnT[h], kn, j, "k")

                    # --- temp (from fp8 x; scale folded into sigmoid) ---
                    tp = ps.tile([1, 512], FP32, tag="tps", bufs=2,
                                 padded_shape=[128, 512], name="tp_temp")
                    first = True
                    for c in range(DCH):
                        for j in range(4):
                            nc.tensor.matmul(
                                tp[:], wt_t[:, c : c + 1],
                                xt8[:, c, j * 512 : (j + 1) * 512],
                                start=first,
                                stop=(c == DCH - 1 and j == 3))
                            first = False
                    tsum = pa.tile([1, 1], FP32, tag="tsum")
                    nc.vector.tensor_reduce(tsum[:], tp[:],
                                            axis=mybir.AxisListType.X,
                                            op=ALU.add)
                    sig = pa.tile([1, 1], FP32, tag="sig")
                    nc.scalar.activation(sig[:], tsum[:], AF.Sigmoid,
                                         bias=bt_t[:],
                                         scale=1.0 / (S * WSCALE))
                    temp = pa.tile([1, 1], FP32, tag="temp")
                    nc.vector.tensor_scalar_add(temp[:], sig[:], 0.5)
                    invt = pa.tile([1, 1], FP32, tag="invt")
                    nc.vector.reciprocal(invt[:], temp[:])
                    nc.gpsimd.partition_broadcast(invt128[:], invt[:])

                    # --- Q projection -> QnT (1/temp folded in) ---
                    for j, pt in proj_fp8(xt8, Wq, bias_t.get("bq"),
                                          RC, wpa, "w8"):
                        qn = pa.tile([128, D], FP16, tag="qn", name="qn",
                                     bufs=3)
                        normalize_pair(pa, pt, qn, invt128[:, 0:1])
                        transpose_to_heads(lambda h: QnT[h], qn, j, "q")

                    # --- gate (fp16, query slice only, resident) ---
                    for n in range(2):
                        for j, pt in proj_fp16_half(
                                xt16, Wg, bias_t.get("bg"), n, RC, wpa,
                                "wg16h"):
                            nc.scalar.activation(
                                gate16[:, j, n * 512 : (n + 1) * 512],
                                pt[:], AF.Sigmoid)

                # ---- late persistent tiles (group phase) ----
                V8 = pp.tile([128, KC, NH, 66], FP8, tag="v8")
                nc.gpsimd.memset(V8[:, :, :, 64:66], 1.0)
                attnT = pp.tile([128, HP, RS], FP16, tag="attnT")

                # ============ selection / attention bodies ============
                def selection_stages(gi, gp):
                    """returns 4 issue-stage closures for group gi's
                    threshold search (2 count iterations)."""
                    heads = list(range(gi * G, (gi + 1) * G))
                    nt = G * RC
                    st = {}

                    def bracket_update(it):
                        acc, st_t = st["acc"], st["st_t"]
                        st_lo, st_hi = st["st_lo"], st["st_hi"]
                        st_clo, st_chi = st["st_clo"], st["st_chi"]
                        islo = gp.tile([128, nt], U8, tag="islo", bufs=2)
                        nc.vector.tensor_scalar(
                            out=islo[:], in0=acc[:], scalar1=cfg.ATGT,
                            scalar2=None, op0=ALU.is_ge)
                        nc.vector.copy_predicated(st_lo[:], islo[:], st_t[:])
                        nc.vector.copy_predicated(st_clo[:], islo[:], acc[:])
                        ishi = gp.tile([128, nt], U8, tag="ishi", bufs=2)
                        nc.vector.tensor_scalar(
                            out=ishi[:], in0=acc[:], scalar1=cfg.ATGT,
                            scalar2=None, op0=ALU.is_lt)
                        nc.vector.copy_predicated(st_hi[:], ishi[:], st_t[:])
                        nc.vector.copy_predicated(st_chi[:], ishi[:], acc[:])
                        tnew = gp.tile([128, nt], FP32, tag="tnew", bufs=2)
                        if it == 0:
                            nc.vector.tensor_scalar(
                                out=tnew[:], in0=acc[:], scalar1=cfg.ATGT,
                                scalar2=1.0 / cfg.slope0, op0=ALU.subtract,
                                op1=ALU.mult)
                            nc.vector.tensor_add(tnew[:], tnew[:], st_t[:])
                        else:
                            den = gp.tile([128, nt], FP32, tag="den",
                                          bufs=2)
                            nc.vector.tensor_sub(den[:], st_clo[:],
                                                 st_chi[:])
                            nc.vector.tensor_scalar_max(den[:], den[:], 1.0)
                            rden = gp.tile([128, nt], FP32, tag="rden",
                                           bufs=2)
                            nc.vector.reciprocal(rden[:], den[:])
                            nc.vector.tensor_scalar(
                                out=tnew[:], in0=st_clo[:],
                                scalar1=cfg.ATGT,
                                scalar2=None, op0=ALU.subtract)
                            nc.vector.tensor_mul(tnew[:], tnew[:], rden[:])
                            wid = gp.tile([128, nt], FP32, tag="wid",
                                          bufs=2)
                            nc.vector.tensor_sub(wid[:], st_hi[:], st_lo[:])
                            nc.vector.tensor_mul(tnew[:], tnew[:], wid[:])
                            nc.vector.tensor_add(tnew[:], tnew[:], st_lo[:])
                        nc.vector.tensor_tensor(tnew[:], tnew[:], st_lo[:],
                                                ALU.max)
                        nc.vector.tensor_tensor(tnew[:], tnew[:], st_hi[:],
                                                ALU.min)
                        iseq = gp.tile([128, nt], U8, tag="iseq", bufs=2)
                        nc.vector.tensor_scalar(
                            out=iseq[:], in0=acc[:], scalar1=cfg.ATGT,
                            scalar2=None, op0=ALU.not_equal)
                        nc.vector.copy_predicated(st_t[:], iseq[:], tnew[:])

                    def s0():
                        nt0 = gp.tile([128, 1], FP32, tag="nt0")
                        nc.vector.memset(nt0[:], -0.1)
                        for nm, val in (("st_t", 0.1), ("st_lo", -2.1),
                                        ("st_hi", 2.1), ("st_clo", float(SS)),
                                        ("st_chi", float(-SS))):
                            t = gp.tile([128, nt], FP32, tag=nm, name=nm)
                            nc.vector.memset(t[:], val)
                            st[nm] = t
                        st["nt0"] = nt0
                        st["acc"] = gp.tile([128, nt], FP32, tag="acc",
                                            name="acc")
                        s16 = {}
                        for hi_, h in enumerate(heads):
                            for j in range(RC):
                                sp_ = ps512(f"selp_{hi_}_{j}")
                                nc.tensor.matmul(
                                    sp_[:],
                                    QnT[h][0:64, j * 128 : (j + 1) * 128],
                                    KnT[h][0:64, 1 : S : cfg.SUB],
                                    start=True, stop=True)
                                srow = gp.tile([128, SS], FP16,
                                               tag=f"s16_{hi_}_{j}",
                                               name=f"s16_{hi_}_{j}")
                                nc.scalar.activation(srow[:], sp_[:],
                                                     AF.Copy)
                                s16[(hi_, j)] = srow
                        st["s16"] = s16

                    def s1():  # it0 counts on ACT (Sign, acc space)
                        for hi_, h in enumerate(heads):
                            for j in range(RC):
                                col = hi_ * RC + j
                                scr = gp.tile([128, SS], FP8, tag="scr8",
                                              bufs=2, name="scr8")
                                nc.scalar.activation(
                                    scr[:], st["s16"][(hi_, j)][:], AF.Sign,
                                    bias=st["nt0"][:, 0:1],
                                    accum_out=st["acc"][:, col : col + 1])

                    def s2():  # it0 bracket + it1 counts on DVE
                        bracket_update(0)
                        for hi_, h in enumerate(heads):
                            for j in range(RC):
                                col = hi_ * RC + j
                                scr = gp.tile([128, SS], FP8,
                                              tag="scr8", bufs=2,
                                              name="scr8d")
                                nc.vector.tensor_scalar(
                                    out=scr[:], in0=st["s16"][(hi_, j)][:],
                                    scalar1=st["st_t"][:, col : col + 1],
                                    scalar2=None, op0=ALU.is_ge,
                                    op1=ALU.add,
                                    accum_out=st["acc"][:, col : col + 1])
                        nc.vector.tensor_scalar(
                            out=st["acc"][:], in0=st["acc"][:], scalar1=2.0,
                            scalar2=float(-SS), op0=ALU.mult, op1=ALU.add)

                    def s3():  # final bracket + tneg -> QnT rows
                        bracket_update(1)
                        tneg = gp.tile([128, nt], FP16, tag="tneg")
                        nc.vector.tensor_scalar(
                            out=tneg[:], in0=st["st_t"][:], scalar1=-1.0,
                            scalar2=None, op0=ALU.mult)
                        ttp = ps.tile([nt, 128], FP16, tag="tps", bufs=2,
                                      padded_shape=[128, 512], name="ttp")
                        nc.tensor.transpose(ttp[:], tneg[:], ident[:])
                        tnT = gp.tile([nt, 128], FP16, tag="tnT")
                        nc.scalar.activation(tnT[:], ttp[:], AF.Copy)
                        for hi_, h in enumerate(heads):
                            for j in range(RC):
                                col = hi_ * RC + j
                                nc.sync.dma_start(
                                    QnT[h][64:65, j * 128 : (j + 1) * 128],
                                    tnT[col : col + 1, :])

                    return [s0, s1, s2, s3]

                def attention_heads(gi):
                    return [lambda h=h: attention_one(h)
                            for h in range(gi * G, (gi + 1) * G)]

                def attention_one(h):
                    if True:
                        avp = ps.tile([65, RS], FP32, tag="avp", bufs=2,
                                      padded_shape=[128, 512], name="avp")
                        for kcp in range(KC // 2):
                            em16 = pp.tile([128, 2, RS], FP16, tag="em16",
                                           bufs=3, name="em16")
                            stp = ps.tile([128, 2, RS], FP32, tag="pt",
                                          bufs=2,
                                          padded_shape=[128, 2, 512],
                                          name="stp")
                            for sub in range(2):
                                kc = 2 * kcp + sub
                                nc.tensor.matmul(
                                    stp[:, sub, :],
                                    KnT[h][:, kc * 128 : (kc + 1) * 128],
                                    QnT[h][:], start=True, stop=True)
                            e16 = pp.tile([128, 2, RS], FP16, tag="e16",
                                          bufs=2, name="e16")
                            nc.scalar.activation(e16[:], stp[:], AF.Exp)
                            if kcp % 8 in (2, 5, 7):
                                # GPSIMD 2-op mask (SBUF-only)
                                nc.gpsimd.tensor_scalar(
                                    out=em16[:], in0=e16[:], scalar1=1.0,
                                    scalar2=None, op0=ALU.is_ge)
                                nc.gpsimd.tensor_tensor(
                                    em16[:], em16[:], e16[:], ALU.mult)
                            else:
                                nc.vector.scalar_tensor_tensor(
                                    out=em16[:], in0=e16[:],
                                    scalar=1.0, in1=e16[:],
                                    op0=ALU.is_ge, op1=ALU.mult)
                            for sub in range(2):
                                kc = 2 * kcp + sub
                                nc.tensor.matmul(
                                    avp[:],
                                    V8[:, kc, h, 0:65],
                                    em16[:, sub, :],
                                    start=(kc == 0), stop=(kc == KC - 1))
                        # normalize: attnT = avp[0:64] * (1/z); z >= 1 by
                        # construction (the max score always passes t)
                        zrec = pp.tile([1, RS], FP32, tag="zrec", bufs=2,
                                       name="zrec")
                        nc.vector.reciprocal(zrec[:], avp[64:65, :])
                        zrep = pp.tile([64, RS], FP32, tag="zrep", bufs=2,
                                       name="zrep")
                        nc.gpsimd.partition_broadcast(zrep[:], zrec[:])
                        nc.vector.tensor_tensor(
                            attnT[(h % 2) * 64 : (h % 2) * 64 + 64,
                                  h // 2, :],
                            avp[0:64, :], zrep[:], ALU.mult)

                # ===== pipeline: selection(g) stages | attention(g-1) ====
                with (
                    tc.tile_pool(name="poolG0", bufs=1) as gp0,
                    tc.tile_pool(name="poolV", bufs=1) as pv,
                ):
                    stages0 = selection_stages(0, gp0)
                    vgen = proj_fp8(xt8, Wv, bias_t.get("bv"), KC, pv,
                                    "wv8")

                    def vchunks(n):
                        for _ in range(n):
                            j, pt = next(vgen)
                            dst = V8[:, j, :, 0:64]
                            src = pt[:].rearrange("p (h d) -> p h d", h=NH)
                            if j % 2 == 0:
                                nc.scalar.activation(dst, src, AF.Copy)
                            else:
                                nc.vector.tensor_copy(dst, src)

                    for s in stages0:
                        s()
                        vchunks(4)

            # poolX8 closed (xt8 freed)
            for gi in range(1, NG):
                with tc.tile_pool(name=f"poolG{gi}", bufs=1) as gp_:
                    stages = selection_stages(gi, gp_)
                    ah = attention_heads(gi - 1)
                    for s, a in zip(stages, ah):
                        s()
                        a()
            for a in attention_heads(NG - 1):
                a()

            # ================ phase C: out proj + gate ================
            with tc.tile_pool(name="poolC", bufs=1) as pc:
                wo_t = pc.tile([128, HP, D], FP16, tag="wo")
                nc.sync.dma_start(wo_t[:], Wo[:])
                xs_t = pc.tile([128, RC, D], FP32, tag="xs")
                nc.sync.dma_start(xs_t[:], xs[:])
                for j in range(RC):
                    op = pt1024("op_out")
                    for n in range(2):
                        sl = slice(n * 512, (n + 1) * 512)
                        for p in range(HP):
                            nc.tensor.matmul(
                                op[:, sl],
                                attnT[:, p, j * 128 : (j + 1) * 128],
                                wo_t[:, p, sl],
                                start=(p == 0),
                                stop=(p == HP - 1 and not with_bias))
                        if with_bias:
                            nc.tensor.matmul(
                                op[:, sl], ones_h[:], bias_t["bo"][:, sl],
                                start=False, stop=True)
                    dd = pc.tile([128, D], FP32, tag="dd", bufs=2,
                                 name="dd")
                    nc.vector.tensor_sub(dd[:], op[:], xs_t[:, j, :])
                    nc.vector.tensor_mul(dd[:], dd[:], gate16[:, j, :])
                    oo = pc.tile([128, D], FP32, tag="oo", bufs=2,
                                 name="oo")
                    nc.gpsimd.tensor_add(oo[:], dd[:], xs_t[:, j, :])
                    nc.sync.dma_start(out[:, j, :], oo[:])

    nc.finalize()
    return nc


# ---------------------------------------------------------------------------
_NC_CACHE = {}
LAST_EXEC_NS = None
LAST_RESULTS = None


def _get_nc(with_bias: bool):
    key = bool(with_bias)
    if key not in _NC_CACHE:
        _NC_CACHE[key] = build(Cfg(), key)
    return _NC_CACHE[key]


def _pack_core_inputs(x, Wq, bq, Wk, bk, Wv, bv, Wo, bo, Wt, bt, Wg, bg,
                      b, r0, cfg, with_bias, fp8):
    S, D, RS, DCH, HP = cfg.S, cfg.D, cfg.RS, cfg.DCH, cfg.HP
    xb = x[b]
    xt = np.ascontiguousarray(
        np.roll(xb.T, -r0, axis=1).reshape(DCH, 128, S).transpose(1, 0, 2))
    xss = np.ascontiguousarray(
        xb[r0 : r0 + RS].reshape(cfg.RC, 128, D).transpose(1, 0, 2))

    def wpack(W, dt, scale=1.0):
        return np.ascontiguousarray(
            (W * scale).reshape(DCH, 128, D).transpose(1, 0, 2)).astype(dt)

    m = {
        "x8T": xt.astype(fp8),
        "x16T": np.ascontiguousarray(xt[:, :, 0:RS]).astype(np.float16),
        "xs": xss.astype(np.float32),
        "Wq": wpack(Wq, fp8, WSCALE),
        "Wk": wpack(Wk, fp8, WSCALE),
        "Wv": wpack(Wv, fp8, WSCALE),
        "Wg": wpack(Wg, np.float16),
        "Wo": np.ascontiguousarray(
            (Wo / WSCALE).reshape(HP, 128, D).transpose(1, 0, 2))
            .astype(np.float16),
        "Wt": np.ascontiguousarray(
            Wt.reshape(DCH, 128).T * WSCALE).astype(fp8),
        "bt": bt.reshape(1, 1).astype(np.float32),
    }
    if with_bias:
        m["bq"] = (bq * WSCALE).reshape(1, D).astype(np.float16)
        m["bk"] = (bk * WSCALE).reshape(1, D).astype(np.float16)
        m["bv"] = (bv * WSCALE).reshape(1, D).astype(np.float16)
        m["bg"] = bg.reshape(1, D).astype(np.float16)
        m["bo"] = bo.reshape(1, D).astype(np.float16)
    return m


def kernel(**inputs):
    from concourse.bass_utils import run_bass_kernel_spmd
    cfg = Cfg()
    fp8 = mybir.dt.np(FP8)
    x = np.asarray(inputs["x"], np.float32)
    B, S, D = x.shape
    args = [np.asarray(inputs[k]) for k in
            ("Wq", "bq", "Wk", "bk", "Wv", "bv", "Wo", "bo", "Wt", "bt",
             "Wg", "bg")]
    with_bias = any(np.any(np.asarray(inputs[k])) for k in
                    ("bq", "bk", "bv", "bg", "bo"))
    nc = _get_nc(with_bias)
    in_maps = []
    for c in range(8):
        b, q = c // 4, c % 4
        in_maps.append(_pack_core_inputs(
            x, *args, b, q * cfg.RS, cfg, with_bias, fp8))
    trace = bool(int(os.environ.get("KERNEL_TRACE", "0")))
    res = run_bass_kernel_spmd(nc, in_maps, core_ids=list(range(8)),
                               trace=trace)
    global LAST_EXEC_NS, LAST_RESULTS
    LAST_EXEC_NS = res.exec_time_ns
    LAST_RESULTS = res
    out = np.empty((B, S, D), np.float32)
    for c in range(8):
        b, q = c // 4, c % 4
        o = res.results[c]["out"]  # [128, RC, D]
        out[b, q * cfg.RS : (q + 1) * cfg.RS] = \
            o.transpose(1, 0, 2).reshape(cfg.RS, D)
    return out


# revision 53
# speedup vs baseline: 2.2737x; 2.2737x over previous
"""EvolvedAttention Trainium2 Bass kernel (v2).

Full inputs -> full output. Sharding: 8 cores = 2 batches x 4 query-row
slices. Each core computes K/V/attention for its (batch, row-slice) with
all 16 heads; host slices inputs and concatenates row-slice outputs.

v2 design (from ntff trace of v1: DVE 73% busy on top-k counting, PE 38%
and cold):
  - Q/K/V projections in fp8e4 + DoubleRow (weights x32 host-side, folded
    back via Wo/32; cosine normalization cancels the scale for q/k).
  - gate/temp/Wo in fp16.
  - KnT (head-major [65, S], ones row for the threshold trick) and the
    gate stay SBUF-resident; no DRAM staging.
  - top-k threshold found on 4x-subsampled keys (strided matmul rhs),
    3 count-iterations split across ACT (Sign+accum), GPSIMD and DVE,
    bracketed false-position in "acc" space (acc = #ge - #lt).
  - scores recomputed transposed with threshold folded in (K=65), exp on
    ACT PSUM->fp8, mask on GPSIMD (em8 = [z>=0]*e8), AV in fp8 DoubleRow
    with a ones column in V8 for the softmax denominator.
  - selection of group g pipelines against attention of group g-1; the
    V projection fills the group-0 selection bubble.
"""

import os
import numpy as np

import concourse.bass as bass
import concourse.mybir as mybir
import concourse.tile as tile
from concourse import bacc

FP32 = mybir.dt.float32
FP16 = mybir.dt.float16
FP8 = mybir.dt.float8e4
U8 = mybir.dt.uint8
AF = mybir.ActivationFunctionType
ALU = mybir.AluOpType
DR = mybir.MatmulPerfMode.DoubleRow

WSCALE = 32.0


class Cfg:
    def __init__(self):
        self.S = 2048
        self.D = 1024
        self.NH = 16
        self.DH = 64
        self.RS = 512
        self.KK = self.S // 4          # top-k
        self.SUB = 4                   # key subsample for threshold search
        self.SS = self.S // self.SUB   # sampled keys (512)
        self.DCH = self.D // 128       # 8
        self.KC = self.S // 128        # 16
        self.RC = self.RS // 128       # 4
        self.HP = self.NH // 2         # 8
        self.GROUP = 4
        self.NG = self.NH // self.GROUP
        self.n_sel_iters = 2
        # target in acc space: acc = 2*c - SS, c target = KK/SUB
        self.ATGT = float(2 * (self.KK // self.SUB) - self.SS)  # -256
        self.slope0 = 2.0 * 2.8 * self.SS  # d(acc)/dt estimate


def build(cfg: Cfg, with_bias: bool):
    nc = bacc.Bacc()
    S, D, NH, DH, RS = cfg.S, cfg.D, cfg.NH, cfg.DH, cfg.RS
    DCH, KC, RC, HP = cfg.DCH, cfg.KC, cfg.RC, cfg.HP
    SS, G, NG = cfg.SS, cfg.GROUP, cfg.NG

    x8T = nc.dram_tensor("x8T", [128, DCH, S], FP8, kind="ExternalInput")
    x16T = nc.dram_tensor("x16T", [128, DCH, RS], FP16, kind="ExternalInput")
    xs = nc.dram_tensor("xs", [128, RC, D], FP32, kind="ExternalInput")
    Wq = nc.dram_tensor("Wq", [128, DCH, D], FP8, kind="ExternalInput")
    Wk = nc.dram_tensor("Wk", [128, DCH, D], FP8, kind="ExternalInput")
    Wv = nc.dram_tensor("Wv", [128, DCH, D], FP8, kind="ExternalInput")
    Wg = nc.dram_tensor("Wg", [128, DCH, D], FP16, kind="ExternalInput")
    Wo = nc.dram_tensor("Wo", [128, HP, D], FP16, kind="ExternalInput")
    Wt = nc.dram_tensor("Wt", [128, DCH], FP8, kind="ExternalInput")
    bt = nc.dram_tensor("bt", [1, 1], FP32, kind="ExternalInput")
    if with_bias:
        bq = nc.dram_tensor("bq", [1, D], FP16, kind="ExternalInput")
        bk = nc.dram_tensor("bk", [1, D], FP16, kind="ExternalInput")
        bv = nc.dram_tensor("bv", [1, D], FP16, kind="ExternalInput")
        bg = nc.dram_tensor("bg", [1, D], FP16, kind="ExternalInput")
        bo = nc.dram_tensor("bo", [1, D], FP16, kind="ExternalInput")
    out = nc.dram_tensor("out", [128, RC, D], FP32, kind="ExternalOutput")

    with tile.TileContext(nc) as tc:
        with (
            tc.tile_pool(name="persist", bufs=1) as pp,
            tc.tile_pool(name="psum", bufs=2, space="PSUM") as ps,
        ):
            # ---------------- persistent tiles (phase A) ----------------
            ident = pp.tile([128, 128], FP16, tag="ident")
            from concourse.masks import make_identity
            make_identity(nc, ident[:])
            ones_h = pp.tile([1, 128], FP16, tag="ones_h")
            nc.vector.memset(ones_h[:], 1.0)
            KnT = [pp.tile([65, S], FP16, tag=f"knt{h}", name=f"knt{h}")
                   for h in range(NH)]
            QnT = [pp.tile([65, RS], FP16, tag=f"qnt{h}", name=f"qnt{h}")
                   for h in range(NH)]
            for h in range(NH):
                nc.gpsimd.memset(KnT[h][64:65, :], 1.0)
            gate16 = pp.tile([128, RC, D], FP16, tag="gate16")
            invt128 = pp.tile([128, 1], FP32, tag="invt128")
            bt_t = pp.tile([1, 1], FP32, tag="bt")
            nc.sync.dma_start(bt_t[:], bt[:])
            wt_t = pp.tile([128, DCH], FP8, tag="wt")
            nc.sync.dma_start(wt_t[:], Wt[:])
            bias_t = {}
            if with_bias:
                for nm, dram in (("bq", bq), ("bk", bk), ("bv", bv),
                                 ("bg", bg), ("bo", bo)):
                    t = pp.tile([1, D], FP16, tag=nm, name=f"b_{nm}")
                    nc.sync.dma_start(t[:], dram[:])
                    bias_t[nm] = t

            def pt1024(name):
                """projection psum: [128,1024] (2 banks), ring of 2."""
                return ps.tile([128, 1024], FP32, tag="pt", bufs=2,
                               padded_shape=[128, 1024], name=name)

            def ps512(name, shape=None, dtype=FP32):
                """small psum ring (transposes, sel-scores, gate, temp)."""
                return ps.tile(shape or [128, 512], dtype, tag="tps",
                               bufs=2, padded_shape=[128, 512], name=name)

            # ---------------- helpers ----------------
            def proj_fp8(xt8, w_dram, bias_row, n_chunks, wpool, wtag):
                """fp8 DoubleRow projection; yields (j, pt) with pt a
                [128,1024] psum row-chunk."""
                w = wpool.tile([128, DCH, D], FP8, tag=wtag, name=wtag,
                               bufs=2)
                nc.sync.dma_start(w[:], w_dram[:])
                for j in range(n_chunks):
                    pt = pt1024(f"pt_{wtag}")
                    for n in range(2):
                        sl = slice(n * 512, (n + 1) * 512)
                        for cp in range(DCH // 2):
                            nc.tensor.matmul(
                                pt[:, sl],
                                xt8[:, 2 * cp : 2 * cp + 2,
                                    j * 128 : (j + 1) * 128],
                                w[:, 2 * cp : 2 * cp + 2, sl],
                                start=(cp == 0),
                                stop=(cp == DCH // 2 - 1 and bias_row is None),
                                perf_mode=DR)
                        if bias_row is not None:
                            nc.tensor.matmul(
                                pt[:, sl], ones_h[:], bias_row[:, sl],
                                start=False, stop=True)
                    yield j, pt

            def proj_fp16_half(xt16, w_dram, bias_row, n, n_chunks, wpool,
                               wtag):
                """one 512-wide output half of an fp16 projection; yields
                (j, pt) psum tiles [128,512]."""
                w = wpool.tile([128, DCH, 512], FP16, tag=wtag, name=wtag,
                               bufs=2)
                nc.sync.dma_start(w[:], w_dram[:, :, n * 512 : (n + 1) * 512])
                for j in range(n_chunks):
                    pt = ps512(f"pt_{wtag}{n}")
                    for c in range(DCH):
                        nc.tensor.matmul(
                            pt[:],
                            xt16[:, c, j * 128 : (j + 1) * 128],
                            w[:, c, :],
                            start=(c == 0),
                            stop=(c == DCH - 1 and bias_row is None))
                    if bias_row is not None:
                        nc.tensor.matmul(
                            pt[:], ones_h[:],
                            bias_row[:, n * 512 : (n + 1) * 512],
                            start=False, stop=True)
                    yield j, pt

            def normalize_pair(sp, pt, dst16, extra_scale_ap):
                """cosine-normalize a [128,1024] psum row-chunk into
                dst16 [128, D] fp16."""
                sq = sp.tile([128, D], FP16, tag="sq", name="sq", bufs=3)
                nc.scalar.activation(sq[:], pt[:], AF.Square)
                n2 = sp.tile([128, NH], FP32, tag="n2", name="n2", bufs=3)
                nc.vector.tensor_reduce(
                    n2[:], sq[:].rearrange("p (h d) -> p h d", h=NH),
                    axis=mybir.AxisListType.X, op=ALU.add)
                rec = sp.tile([128, NH], FP32, tag="rec", name="rec", bufs=3)
                nc.vector.tensor_scalar_max(rec[:], n2[:], 1e-12)
                nc.vector.reciprocal(rec[:], rec[:])
                rsq = sp.tile([128, NH], FP32, tag="rsq", name="rsq", bufs=3)
                nc.scalar.activation(rsq[:], rec[:], AF.Sqrt)
                if extra_scale_ap is not None:
                    nc.vector.tensor_scalar(
                        out=rsq[:], in0=rsq[:], scalar1=extra_scale_ap,
                        scalar2=None, op0=ALU.mult)
                nc.vector.tensor_tensor(
                    dst16[:].rearrange("p (h d) -> p h d", h=NH),
                    pt[:].rearrange("p (h d) -> p h d", h=NH),
                    rsq[:].rearrange("p (h o) -> p h o", o=1)
                        .to_broadcast([128, NH, DH]),
                    ALU.mult)

            def transpose_to_heads(dst_of_head, src16, j, who):
                """src16 [128 rows, 1024] -> per-head [64, 128] blocks into
                dst_of_head(h)[0:64, j*128:(j+1)*128]."""
                for p in range(HP):
                    tps = ps.tile([128, 128], FP16, tag="tps", bufs=2,
                                  padded_shape=[128, 512], name=f"tps_{who}")
                    nc.tensor.transpose(
                        tps[:], src16[:, p * 128 : (p + 1) * 128], ident[:])
                    for hh in range(2):
                        h = 2 * p + hh
                        dst = dst_of_head(h)[0:64, j * 128 : (j + 1) * 128]
                        src = tps[hh * 64 : hh * 64 + 64, :]
                        if (p + hh + j) % 2 == 0:
                            nc.scalar.activation(dst, src, AF.Copy)
                        else:
                            nc.vector.tensor_copy(dst, src)

            # ================ phase A ================
            with tc.tile_pool(name="poolX8", bufs=1) as px:
                xt8 = px.tile([128, DCH, S], FP8, tag="xt8")
                nc.sync.dma_start(xt8[:], x8T[:])

                with (
                    tc.tile_pool(name="poolA", bufs=1) as pa,
                    tc.tile_pool(name="wpoolA", bufs=2) as wpa,
                ):
                    xt16 = pa.tile([128, DCH, RS], FP16, tag="xt16")
                    nc.sync.dma_start(xt16[:], x16T[:])

                    # --- K projection -> KnT (resident); transposes run
                    # one chunk behind so PE never waits on normalize ---
                    pend = None
                    for j, pt in proj_fp8(xt8, Wk, bias_t.get("bk"),
                                          KC, wpa, "w8"):
                        kn = pa.tile([128, D], FP16, tag="kn", name="kn",
                                     bufs=3)
                        normalize_pair(pa, pt, kn, None)
                        if pend is not None:
                            transpose_to_heads(lambda h: KnT[h], pend[0],
                                               pend[1], "k")
                        pend = (kn, j)
                    transpose_to_heads(lambda h: KnT[h], pend[0], pend[1],
                                       "k")

                    # --- temp (from fp8 x; scale folded into sigmoid) ---
                    tp = ps.tile([1, 512], FP32, tag="tps", bufs=2,
                                 padded_shape=[128, 512], name="tp_temp")
                    first = True
                    for c in range(DCH):
                        for j in range(4):
                            nc.tensor.matmul(
                                tp[:], wt_t[:, c : c + 1],
                                xt8[:, c, j * 512 : (j + 1) * 512],
                                start=first,
                                stop=(c == DCH - 1 and j == 3))
                            first = False
                    tsum = pa.tile([1, 1], FP32, tag="tsum")
                    nc.vector.tensor_reduce(tsum[:], tp[:],
                                            axis=mybir.AxisListType.X,
                                            op=ALU.add)
                    sig = pa.tile([1, 1], FP32, tag="sig")
                    nc.scalar.activation(sig[:], tsum[:], AF.Sigmoid,
                                         bias=bt_t[:],
                                         scale=1.0 / (S * WSCALE))
                    temp = pa.tile([1, 1], FP32, tag="temp")
                    nc.vector.tensor_scalar_add(temp[:], sig[:], 0.5)
                    invt = pa.tile([1, 1], FP32, tag="invt")
                    nc.vector.reciprocal(invt[:], temp[:])
                    nc.gpsimd.partition_broadcast(invt128[:], invt[:])

                    # --- Q projection -> QnT (1/temp folded in) ---
                    for j, pt in proj_fp8(xt8, Wq, bias_t.get("bq"),
                                          RC, wpa, "w8"):
                        qn = pa.tile([128, D], FP16, tag="qn", name="qn",
                                     bufs=3)
                        normalize_pair(pa, pt, qn, invt128[:, 0:1])
                        transpose_to_heads(lambda h: QnT[h], qn, j, "q")

                    # --- gate (fp16, query slice only, resident) ---
                    for n in range(2):
                        for j, pt in proj_fp16_half(
                                xt16, Wg, bias_t.get("bg"), n, RC, wpa,
                                "wg16h"):
                            nc.scalar.activation(
                                gate16[:, j, n * 512 : (n + 1) * 512],
                                pt[:], AF.Sigmoid)

                # ---- late persistent tiles (group phase) ----
                V8 = pp.tile([128, KC, NH, 66], FP8, tag="v8")
                nc.gpsimd.memset(V8[:, :, :, 64:66], 1.0)
                attnT = pp.tile([128, HP, RS], FP16, tag="attnT")

                # ============ selection / attention bodies ============
                def selection_stages(gi, gp):
                    """returns 4 issue-stage closures for group gi's
                    threshold search (2 count iterations)."""
                    heads = list(range(gi * G, (gi + 1) * G))
                    nt = G * RC
                    st = {}

                    def bracket_update(it):
                        acc, st_t = st["acc"], st["st_t"]
                        st_lo, st_hi = st["st_lo"], st["st_hi"]
                        st_clo, st_chi = st["st_clo"], st["st_chi"]
                        islo = gp.tile([128, nt], U8, tag="islo", bufs=2)
                        nc.vector.tensor_scalar(
                            out=islo[:], in0=acc[:], scalar1=cfg.ATGT,
                            scalar2=None, op0=ALU.is_ge)
                        nc.vector.copy_predicated(st_lo[:], islo[:], st_t[:])
                        nc.vector.copy_predicated(st_clo[:], islo[:], acc[:])
                        ishi = gp.tile([128, nt], U8, tag="ishi", bufs=2)
                        nc.vector.tensor_scalar(
                            out=ishi[:], in0=acc[:], scalar1=cfg.ATGT,
                            scalar2=None, op0=ALU.is_lt)
                        nc.vector.copy_predicated(st_hi[:], ishi[:], st_t[:])
                        nc.vector.copy_predicated(st_chi[:], ishi[:], acc[:])
                        tnew = gp.tile([128, nt], FP32, tag="tnew", bufs=2)
                        if it == 0:
                            nc.vector.tensor_scalar(
                                out=tnew[:], in0=acc[:], scalar1=cfg.ATGT,
                                scalar2=1.0 / cfg.slope0, op0=ALU.subtract,
                                op1=ALU.mult)
                            nc.vector.tensor_add(tnew[:], tnew[:], st_t[:])
                        else:
                            den = gp.tile([128, nt], FP32, tag="den",
                                          bufs=2)
                            nc.vector.tensor_sub(den[:], st_clo[:],
                                                 st_chi[:])
                            nc.vector.tensor_scalar_max(den[:], den[:], 1.0)
                            rden = gp.tile([128, nt], FP32, tag="rden",
                                           bufs=2)
                            nc.vector.reciprocal(rden[:], den[:])
                            nc.vector.tensor_scalar(
                                out=tnew[:], in0=st_clo[:],
                                scalar1=cfg.ATGT,
                                scalar2=None, op0=ALU.subtract)
                            nc.vector.tensor_mul(tnew[:], tnew[:], rden[:])
                            wid = gp.tile([128, nt], FP32, tag="wid",
                                          bufs=2)
                            nc.vector.tensor_sub(wid[:], st_hi[:], st_lo[:])
                            nc.vector.tensor_mul(tnew[:], tnew[:], wid[:])
                            nc.vector.tensor_add(tnew[:], tnew[:], st_lo[:])
                        nc.vector.tensor_tensor(tnew[:], tnew[:], st_lo[:],
                                                ALU.max)
                        nc.vector.tensor_tensor(tnew[:], tnew[:], st_hi[:],
                                                ALU.min)
                        iseq = gp.tile([128, nt], U8, tag="iseq", bufs=2)
                        nc.vector.tensor_scalar(
                            out=iseq[:], in0=acc[:], scalar1=cfg.ATGT,
                            scalar2=None, op0=ALU.not_equal)
                        nc.vector.copy_predicated(st_t[:], iseq[:], tnew[:])

                    def s0():
                        nt0 = gp.tile([128, 1], FP32, tag="nt0")
                        nc.vector.memset(nt0[:], -0.1)
                        for nm, val in (("st_t", 0.1), ("st_lo", -2.1),
                                        ("st_hi", 2.1), ("st_clo", float(SS)),
                                        ("st_chi", float(-SS))):
                            t = gp.tile([128, nt], FP32, tag=nm, name=nm)
                            nc.vector.memset(t[:], val)
                            st[nm] = t
                        st["nt0"] = nt0
                        st["acc"] = gp.tile([128, nt], FP32, tag="acc",
                                            name="acc")
                        s16 = {}
                        for hi_, h in enumerate(heads):
                            for j in range(RC):
                                sp_ = ps512(f"selp_{hi_}_{j}")
                                nc.tensor.matmul(
                                    sp_[:],
                                    QnT[h][0:64, j * 128 : (j + 1) * 128],
                                    KnT[h][0:64, 1 : S : cfg.SUB],
                                    start=True, stop=True)
                                srow = gp.tile([128, SS], FP16,
                                               tag=f"s16_{hi_}_{j}",
                                               name=f"s16_{hi_}_{j}")
                                nc.scalar.activation(srow[:], sp_[:],
                                                     AF.Copy)
                                s16[(hi_, j)] = srow
                        st["s16"] = s16

                    def s1():  # it0 counts on ACT (Sign, acc space)
                        for hi_, h in enumerate(heads):
                            for j in range(RC):
                                col = hi_ * RC + j
                                scr = gp.tile([128, SS], FP8, tag="scr8",
                                              bufs=2, name="scr8")
                                nc.scalar.activation(
                                    scr[:], st["s16"][(hi_, j)][:], AF.Sign,
                                    bias=st["nt0"][:, 0:1],
                                    accum_out=st["acc"][:, col : col + 1])

                    def s2():  # it0 bracket + it1 counts on DVE
                        bracket_update(0)
                        for hi_, h in enumerate(heads):
                            for j in range(RC):
                                col = hi_ * RC + j
                                scr = gp.tile([128, SS], FP8,
                                              tag="scr8", bufs=2,
                                              name="scr8d")
                                nc.vector.tensor_scalar(
                                    out=scr[:], in0=st["s16"][(hi_, j)][:],
                                    scalar1=st["st_t"][:, col : col + 1],
                                    scalar2=None, op0=ALU.is_ge,
                                    op1=ALU.add,
                                    accum_out=st["acc"][:, col : col + 1])
                        nc.vector.tensor_scalar(
                            out=st["acc"][:], in0=st["acc"][:], scalar1=2.0,
                            scalar2=float(-SS), op0=ALU.mult, op1=ALU.add)

                    def s3():  # final bracket + tneg -> QnT rows
                        bracket_update(1)
                        tneg = gp.tile([128, nt], FP16, tag="tneg")
                        nc.vector.tensor_scalar(
                            out=tneg[:], in0=st["st_t"][:], scalar1=-1.0,
                            scalar2=None, op0=ALU.mult)
                        ttp = ps.tile([nt, 128], FP16, tag="tps", bufs=2,
                                      padded_shape=[128, 512], name="ttp")
                        nc.tensor.transpose(ttp[:], tneg[:], ident[:])
                        tnT = gp.tile([nt, 128], FP16, tag="tnT")
                        nc.scalar.activation(tnT[:], ttp[:], AF.Copy)
                        for hi_, h in enumerate(heads):
                            for j in range(RC):
                                col = hi_ * RC + j
                                nc.sync.dma_start(
                                    QnT[h][64:65, j * 128 : (j + 1) * 128],
                                    tnT[col : col + 1, :])

                    return [s0, s1, s2, s3]

                def attention_heads(gi):
                    return [lambda h=h: attention_one(h)
                            for h in range(gi * G, (gi + 1) * G)]

                def attention_one(h):
                    if True:
                        avp = ps.tile([65, RS], FP32, tag="avp", bufs=2,
                                      padded_shape=[128, 512], name="avp")
                        for kcp in range(KC // 2):
                            em8 = pp.tile([128, 2, RS], FP8, tag="em8",
                                          bufs=4, name="em8")
                            stp = ps.tile([128, 2, RS], FP32, tag="pt",
                                          bufs=2,
                                          padded_shape=[128, 2, 512],
                                          name="stp")
                            for sub in range(2):
                                kc = 2 * kcp + sub
                                nc.tensor.matmul(
                                    stp[:, sub, :],
                                    KnT[h][:, kc * 128 : (kc + 1) * 128],
                                    QnT[h][:], start=True, stop=True)
                            e16 = pp.tile([128, 2, RS], FP16, tag="e16",
                                          bufs=2, name="e16")
                            nc.scalar.activation(e16[:], stp[:], AF.Exp)
                            nc.vector.scalar_tensor_tensor(
                                out=em8[:], in0=e16[:],
                                scalar=1.0, in1=e16[:],
                                op0=ALU.is_ge, op1=ALU.mult)
                            nc.tensor.matmul(
                                avp[:],
                                V8[:, 2 * kcp : 2 * kcp + 2, h, 0:65],
                                em8[:, :, :],
                                start=(kcp == 0), stop=(kcp == KC // 2 - 1),
                                perf_mode=DR)
                        # normalize: attnT = avp[0:64] * (1/z); z >= 1 by
                        # construction (the max score always passes t)
                        zrec = pp.tile([1, RS], FP32, tag="zrec", bufs=2,
                                       name="zrec")
                        nc.vector.reciprocal(zrec[:], avp[64:65, :])
                        zrep = pp.tile([64, RS], FP32, tag="zrep", bufs=2,
                                       name="zrep")
                        nc.gpsimd.partition_broadcast(zrep[:], zrec[:])
                        nc.vector.tensor_tensor(
                            attnT[(h % 2) * 64 : (h % 2) * 64 + 64,
                                  h // 2, :],
                            avp[0:64, :], zrep[:], ALU.mult)

                # ===== pipeline: selection(g) stages | attention(g-1) ====
                with (
                    tc.tile_pool(name="poolG0", bufs=1) as gp0,
                    tc.tile_pool(name="poolV", bufs=1) as pv,
                ):
                    stages0 = selection_stages(0, gp0)
                    vgen = proj_fp8(xt8, Wv, bias_t.get("bv"), KC, pv,
                                    "wv8")

                    def vchunks(n):
                        for _ in range(n):
                            j, pt = next(vgen)
                            dst = V8[:, j, :, 0:64]
                            src = pt[:].rearrange("p (h d) -> p h d", h=NH)
                            if j % 2 == 0:
                                nc.scalar.activation(dst, src, AF.Copy)
                            else:
                                nc.vector.tensor_copy(dst, src)

                    for s in stages0:
                        s()
                        vchunks(4)

            # poolX8 closed (xt8 freed)
            for gi in range(1, NG):
                with tc.tile_pool(name=f"poolG{gi}", bufs=1) as gp_:
                    stages = selection_stages(gi, gp_)
                    ah = attention_heads(gi - 1)
                    for s, a in zip(stages, ah):
                        s()
                        a()
            for a in attention_heads(NG - 1):
                a()

            # ================ phase C: out proj + gate ================
            with tc.tile_pool(name="poolC", bufs=1) as pc:
                wo_t = pc.tile([128, HP, D], FP16, tag="wo")
                nc.sync.dma_start(wo_t[:], Wo[:])
                xs_t = pc.tile([128, RC, D], FP32, tag="xs")
                nc.sync.dma_start(xs_t[:], xs[:])
                for j in range(RC):
                    op = pt1024("op_out")
                    for n in range(2):
                        sl = slice(n * 512, (n + 1) * 512)
                        for p in range(HP):
                            nc.tensor.matmul(
                                op[:, sl],
                                attnT[:, p, j * 128 : (j + 1) * 128],
                                wo_t[:, p, sl],
                                start=(p == 0),
                                stop=(p == HP - 1 and not with_bias))
                        if with_bias:
                            nc.tensor.matmul(
                                op[:, sl], ones_h[:], bias_t["bo"][:, sl],
                                start=False, stop=True)
                    dd = pc.tile([128, D], FP32, tag="dd", bufs=2,
                                 name="dd")
                    nc.vector.tensor_sub(dd[:], op[:], xs_t[:, j, :])
                    nc.vector.tensor_mul(dd[:], dd[:], gate16[:, j, :])
                    oo = pc.tile([128, D], FP32, tag="oo", bufs=2,
                                 name="oo")
                    nc.gpsimd.tensor_add(oo[:], dd[:], xs_t[:, j, :])
                    nc.sync.dma_start(out[:, j, :], oo[:])

    nc.finalize()
    return nc


# ---------------------------------------------------------------------------
_NC_CACHE = {}
LAST_EXEC_NS = None
LAST_RESULTS = None


def _get_nc(with_bias: bool):
    key = bool(with_bias)
    if key not in _NC_CACHE:
        _NC_CACHE[key] = build(Cfg(), key)
    return _NC_CACHE[key]


def _pack_core_inputs(x, Wq, bq, Wk, bk, Wv, bv, Wo, bo, Wt, bt, Wg, bg,
                      b, r0, cfg, with_bias, fp8):
    S, D, RS, DCH, HP = cfg.S, cfg.D, cfg.RS, cfg.DCH, cfg.HP
    xb = x[b]
    xt = np.ascontiguousarray(
        np.roll(xb.T, -r0, axis=1).reshape(DCH, 128, S).transpose(1, 0, 2))
    xss = np.ascontiguousarray(
        xb[r0 : r0 + RS].reshape(cfg.RC, 128, D).transpose(1, 0, 2))

    def wpack(W, dt, scale=1.0):
        return np.ascontiguousarray(
            (W * scale).reshape(DCH, 128, D).transpose(1, 0, 2)).astype(dt)

    m = {
        "x8T": xt.astype(fp8),
        "x16T": np.ascontiguousarray(xt[:, :, 0:RS]).astype(np.float16),
        "xs": xss.astype(np.float32),
        "Wq": wpack(Wq, fp8, WSCALE),
        "Wk": wpack(Wk, fp8, WSCALE),
        "Wv": wpack(Wv, fp8, WSCALE),
        "Wg": wpack(Wg, np.float16),
        "Wo": np.ascontiguousarray(
            (Wo / WSCALE).reshape(HP, 128, D).transpose(1, 0, 2))
            .astype(np.float16),
        "Wt": np.ascontiguousarray(
            Wt.reshape(DCH, 128).T * WSCALE).astype(fp8),
        "bt": bt.reshape(1, 1).astype(np.float32),
    }
    if with_bias:
        m["bq"] = (bq * WSCALE).reshape(1, D).astype(np.float16)
        m["bk"] = (bk * WSCALE).reshape(1, D).astype(np.float16)
        m["bv"] = (bv * WSCALE).reshape(1, D).astype(np.float16)
        m["bg"] = bg.reshape(1, D).astype(np.float16)
        m["bo"] = bo.reshape(1, D).astype(np.float16)
    return m


def kernel(**inputs):
    from concourse.bass_utils import run_bass_kernel_spmd
    cfg = Cfg()
    fp8 = mybir.dt.np(FP8)
    x = np.asarray(inputs["x"], np.float32)
    B, S, D = x.shape
    args = [np.asarray(inputs[k]) for k in
            ("Wq", "bq", "Wk", "bk", "Wv", "bv", "Wo", "bo", "Wt", "bt",
             "Wg", "bg")]
    with_bias = any(np.any(np.asarray(inputs[k])) for k in
                    ("bq", "bk", "bv", "bg", "bo"))
    nc = _get_nc(with_bias)
    in_maps = []
    for c in range(8):
        b, q = c // 4, c % 4
        in_maps.append(_pack_core_inputs(
            x, *args, b, q * cfg.RS, cfg, with_bias, fp8))
    trace = bool(int(os.environ.get("KERNEL_TRACE", "0")))
    res = run_bass_kernel_spmd(nc, in_maps, core_ids=list(range(8)),
                               trace=trace)
    global LAST_EXEC_NS, LAST_RESULTS
    LAST_EXEC_NS = res.exec_time_ns
    LAST_RESULTS = res
    out = np.empty((B, S, D), np.float32)
    for c in range(8):
        b, q = c // 4, c % 4
        o = res.results[c]["out"]  # [128, RC, D]
        out[b, q * cfg.RS : (q + 1) * cfg.RS] = \
            o.transpose(1, 0, 2).reshape(cfg.RS, D)
    return out


# revision 54
# speedup vs baseline: 2.2769x; 1.0014x over previous
"""EvolvedAttention Trainium2 Bass kernel (v2).

Full inputs -> full output. Sharding: 8 cores = 2 batches x 4 query-row
slices. Each core computes K/V/attention for its (batch, row-slice) with
all 16 heads; host slices inputs and concatenates row-slice outputs.

v2 design (from ntff trace of v1: DVE 73% busy on top-k counting, PE 38%
and cold):
  - Q/K/V projections in fp8e4 + DoubleRow (weights x32 host-side, folded
    back via Wo/32; cosine normalization cancels the scale for q/k).
  - gate/temp/Wo in fp16.
  - KnT (head-major [65, S], ones row for the threshold trick) and the
    gate stay SBUF-resident; no DRAM staging.
  - top-k threshold found on 4x-subsampled keys (strided matmul rhs),
    3 count-iterations split across ACT (Sign+accum), GPSIMD and DVE,
    bracketed false-position in "acc" space (acc = #ge - #lt).
  - scores recomputed transposed with threshold folded in (K=65), exp on
    ACT PSUM->fp8, mask on GPSIMD (em8 = [z>=0]*e8), AV in fp8 DoubleRow
    with a ones column in V8 for the softmax denominator.
  - selection of group g pipelines against attention of group g-1; the
    V projection fills the group-0 selection bubble.
"""

import os
import numpy as np

import concourse.bass as bass
import concourse.mybir as mybir
import concourse.tile as tile
from concourse import bacc

FP32 = mybir.dt.float32
FP16 = mybir.dt.float16
FP8 = mybir.dt.float8e4
U8 = mybir.dt.uint8
AF = mybir.ActivationFunctionType
ALU = mybir.AluOpType
DR = mybir.MatmulPerfMode.DoubleRow

WSCALE = 32.0


class Cfg:
    def __init__(self):
        self.S = 2048
        self.D = 1024
        self.NH = 16
        self.DH = 64
        self.RS = 512
        self.KK = self.S // 4          # top-k
        self.SUB = 4                   # key subsample for threshold search
        self.SS = self.S // self.SUB   # sampled keys (512)
        self.DCH = self.D // 128       # 8
        self.KC = self.S // 128        # 16
        self.RC = self.RS // 128       # 4
        self.HP = self.NH // 2         # 8
        self.GROUP = 4
        self.NG = self.NH // self.GROUP
        self.n_sel_iters = 2
        # target in acc space: acc = 2*c - SS, c target = KK/SUB
        self.ATGT = float(2 * (self.KK // self.SUB) - self.SS)  # -256
        self.slope0 = 2.0 * 2.8 * self.SS  # d(acc)/dt estimate


def build(cfg: Cfg, with_bias: bool):
    nc = bacc.Bacc()
    S, D, NH, DH, RS = cfg.S, cfg.D, cfg.NH, cfg.DH, cfg.RS
    DCH, KC, RC, HP = cfg.DCH, cfg.KC, cfg.RC, cfg.HP
    SS, G, NG = cfg.SS, cfg.GROUP, cfg.NG

    x8T = nc.dram_tensor("x8T", [128, DCH, S], FP8, kind="ExternalInput")
    x16T = nc.dram_tensor("x16T", [128, DCH, RS], FP16, kind="ExternalInput")
    xs = nc.dram_tensor("xs", [128, RC, D], FP32, kind="ExternalInput")
    Wq = nc.dram_tensor("Wq", [128, DCH, D], FP8, kind="ExternalInput")
    Wk = nc.dram_tensor("Wk", [128, DCH, D], FP8, kind="ExternalInput")
    Wv = nc.dram_tensor("Wv", [128, DCH, D], FP8, kind="ExternalInput")
    Wg = nc.dram_tensor("Wg", [128, DCH, D], FP16, kind="ExternalInput")
    Wo = nc.dram_tensor("Wo", [128, HP, D], FP16, kind="ExternalInput")
    Wt = nc.dram_tensor("Wt", [128, DCH], FP8, kind="ExternalInput")
    bt = nc.dram_tensor("bt", [1, 1], FP32, kind="ExternalInput")
    if with_bias:
        bq = nc.dram_tensor("bq", [1, D], FP16, kind="ExternalInput")
        bk = nc.dram_tensor("bk", [1, D], FP16, kind="ExternalInput")
        bv = nc.dram_tensor("bv", [1, D], FP16, kind="ExternalInput")
        bg = nc.dram_tensor("bg", [1, D], FP16, kind="ExternalInput")
        bo = nc.dram_tensor("bo", [1, D], FP16, kind="ExternalInput")
    out = nc.dram_tensor("out", [128, RC, D], FP32, kind="ExternalOutput")

    with tile.TileContext(nc) as tc:
        with (
            tc.tile_pool(name="persist", bufs=1) as pp,
            tc.tile_pool(name="psum", bufs=2, space="PSUM") as ps,
        ):
            # ---------------- persistent tiles (phase A) ----------------
            ident = pp.tile([128, 128], FP16, tag="ident")
            from concourse.masks import make_identity
            make_identity(nc, ident[:])
            ones_h = pp.tile([1, 128], FP16, tag="ones_h")
            nc.vector.memset(ones_h[:], 1.0)
            KnT = [pp.tile([65, S], FP16, tag=f"knt{h}", name=f"knt{h}")
                   for h in range(NH)]
            QnT = [pp.tile([65, RS], FP16, tag=f"qnt{h}", name=f"qnt{h}")
                   for h in range(NH)]
            for h in range(NH):
                nc.gpsimd.memset(KnT[h][64:65, :], 1.0)
            gate16 = pp.tile([128, RC, D], FP16, tag="gate16")
            invt128 = pp.tile([128, 1], FP32, tag="invt128")
            bt_t = pp.tile([1, 1], FP32, tag="bt")
            nc.sync.dma_start(bt_t[:], bt[:])
            wt_t = pp.tile([128, DCH], FP8, tag="wt")
            nc.sync.dma_start(wt_t[:], Wt[:])
            bias_t = {}
            if with_bias:
                for nm, dram in (("bq", bq), ("bk", bk), ("bv", bv),
                                 ("bg", bg), ("bo", bo)):
                    t = pp.tile([1, D], FP16, tag=nm, name=f"b_{nm}")
                    nc.sync.dma_start(t[:], dram[:])
                    bias_t[nm] = t

            def pt1024(name):
                """projection psum: [128,1024] (2 banks), ring of 2."""
                return ps.tile([128, 1024], FP32, tag="pt", bufs=2,
                               padded_shape=[128, 1024], name=name)

            def ps512(name, shape=None, dtype=FP32):
                """small psum ring (transposes, sel-scores, gate, temp)."""
                return ps.tile(shape or [128, 512], dtype, tag="tps",
                               bufs=2, padded_shape=[128, 512], name=name)

            # ---------------- helpers ----------------
            def proj_fp8(xt8, w_dram, bias_row, n_chunks, wpool, wtag):
                """fp8 DoubleRow projection; yields (j, pt) with pt a
                [128,1024] psum row-chunk."""
                w = wpool.tile([128, DCH, D], FP8, tag=wtag, name=wtag,
                               bufs=2)
                nc.sync.dma_start(w[:], w_dram[:])
                for j in range(n_chunks):
                    pt = pt1024(f"pt_{wtag}")
                    for n in range(2):
                        sl = slice(n * 512, (n + 1) * 512)
                        for cp in range(DCH // 2):
                            nc.tensor.matmul(
                                pt[:, sl],
                                xt8[:, 2 * cp : 2 * cp + 2,
                                    j * 128 : (j + 1) * 128],
                                w[:, 2 * cp : 2 * cp + 2, sl],
                                start=(cp == 0),
                                stop=(cp == DCH // 2 - 1 and bias_row is None),
                                perf_mode=DR)
                        if bias_row is not None:
                            nc.tensor.matmul(
                                pt[:, sl], ones_h[:], bias_row[:, sl],
                                start=False, stop=True)
                    yield j, pt

            def proj_fp16_half(xt16, w_dram, bias_row, n, n_chunks, wpool,
                               wtag):
                """one 512-wide output half of an fp16 projection; yields
                (j, pt) psum tiles [128,512]."""
                w = wpool.tile([128, DCH, 512], FP16, tag=wtag, name=wtag,
                               bufs=2)
                nc.sync.dma_start(w[:], w_dram[:, :, n * 512 : (n + 1) * 512])
                for j in range(n_chunks):
                    pt = ps512(f"pt_{wtag}{n}")
                    for c in range(DCH):
                        nc.tensor.matmul(
                            pt[:],
                            xt16[:, c, j * 128 : (j + 1) * 128],
                            w[:, c, :],
                            start=(c == 0),
                            stop=(c == DCH - 1 and bias_row is None))
                    if bias_row is not None:
                        nc.tensor.matmul(
                            pt[:], ones_h[:],
                            bias_row[:, n * 512 : (n + 1) * 512],
                            start=False, stop=True)
                    yield j, pt

            def normalize_pair(sp, pt, dst16, extra_scale_ap):
                """cosine-normalize a [128,1024] psum row-chunk into
                dst16 [128, D] fp16."""
                sq = sp.tile([128, D], FP16, tag="sq", name="sq", bufs=3)
                nc.scalar.activation(sq[:], pt[:], AF.Square)
                n2 = sp.tile([128, NH], FP32, tag="n2", name="n2", bufs=3)
                nc.vector.tensor_reduce(
                    n2[:], sq[:].rearrange("p (h d) -> p h d", h=NH),
                    axis=mybir.AxisListType.X, op=ALU.add)
                rec = sp.tile([128, NH], FP32, tag="rec", name="rec", bufs=3)
                nc.vector.tensor_scalar_max(rec[:], n2[:], 1e-12)
                nc.vector.reciprocal(rec[:], rec[:])
                rsq = sp.tile([128, NH], FP32, tag="rsq", name="rsq", bufs=3)
                nc.scalar.activation(rsq[:], rec[:], AF.Sqrt)
                if extra_scale_ap is not None:
                    nc.vector.tensor_scalar(
                        out=rsq[:], in0=rsq[:], scalar1=extra_scale_ap,
                        scalar2=None, op0=ALU.mult)
                nc.vector.tensor_tensor(
                    dst16[:].rearrange("p (h d) -> p h d", h=NH),
                    pt[:].rearrange("p (h d) -> p h d", h=NH),
                    rsq[:].rearrange("p (h o) -> p h o", o=1)
                        .to_broadcast([128, NH, DH]),
                    ALU.mult)

            def transpose_to_heads(dst_of_head, src16, j, who):
                """src16 [128 rows, 1024] -> per-head [64, 128] blocks into
                dst_of_head(h)[0:64, j*128:(j+1)*128]."""
                for p in range(HP):
                    tps = ps.tile([128, 128], FP16, tag="tps", bufs=2,
                                  padded_shape=[128, 512], name=f"tps_{who}")
                    nc.tensor.transpose(
                        tps[:], src16[:, p * 128 : (p + 1) * 128], ident[:])
                    for hh in range(2):
                        h = 2 * p + hh
                        dst = dst_of_head(h)[0:64, j * 128 : (j + 1) * 128]
                        src = tps[hh * 64 : hh * 64 + 64, :]
                        if (p + hh + j) % 2 == 0:
                            nc.scalar.activation(dst, src, AF.Copy)
                        else:
                            nc.vector.tensor_copy(dst, src)

            # ================ phase A ================
            with tc.tile_pool(name="poolX8", bufs=1) as px:
                xt8 = px.tile([128, DCH, S], FP8, tag="xt8")
                nc.sync.dma_start(xt8[:], x8T[:])

                with (
                    tc.tile_pool(name="poolA", bufs=1) as pa,
                    tc.tile_pool(name="wpoolA", bufs=2) as wpa,
                ):
                    xt16 = pa.tile([128, DCH, RS], FP16, tag="xt16")
                    nc.sync.dma_start(xt16[:], x16T[:])

                    # --- K projection -> KnT (resident); transposes run
                    # one chunk behind so PE never waits on normalize ---
                    pend = None
                    for j, pt in proj_fp8(xt8, Wk, bias_t.get("bk"),
                                          KC, wpa, "w8"):
                        kn = pa.tile([128, D], FP16, tag="kn", name="kn",
                                     bufs=3)
                        normalize_pair(pa, pt, kn, None)
                        if pend is not None:
                            transpose_to_heads(lambda h: KnT[h], pend[0],
                                               pend[1], "k")
                        pend = (kn, j)
                    transpose_to_heads(lambda h: KnT[h], pend[0], pend[1],
                                       "k")

                    # --- temp (from fp8 x; scale folded into sigmoid) ---
                    tp = ps.tile([1, 512], FP32, tag="tps", bufs=2,
                                 padded_shape=[128, 512], name="tp_temp")
                    first = True
                    for c in range(DCH):
                        for j in range(4):
                            nc.tensor.matmul(
                                tp[:], wt_t[:, c : c + 1],
                                xt8[:, c, j * 512 : (j + 1) * 512],
                                start=first,
                                stop=(c == DCH - 1 and j == 3))
                            first = False
                    tsum = pa.tile([1, 1], FP32, tag="tsum")
                    nc.vector.tensor_reduce(tsum[:], tp[:],
                                            axis=mybir.AxisListType.X,
                                            op=ALU.add)
                    sig = pa.tile([1, 1], FP32, tag="sig")
                    nc.scalar.activation(sig[:], tsum[:], AF.Sigmoid,
                                         bias=bt_t[:],
                                         scale=1.0 / (S * WSCALE))
                    temp = pa.tile([1, 1], FP32, tag="temp")
                    nc.vector.tensor_scalar_add(temp[:], sig[:], 0.5)
                    invt = pa.tile([1, 1], FP32, tag="invt")
                    nc.vector.reciprocal(invt[:], temp[:])
                    nc.gpsimd.partition_broadcast(invt128[:], invt[:])

                    # --- Q projection -> QnT (1/temp folded in) ---
                    pend = None
                    for j, pt in proj_fp8(xt8, Wq, bias_t.get("bq"),
                                          RC, wpa, "w8"):
                        qn = pa.tile([128, D], FP16, tag="qn", name="qn",
                                     bufs=3)
                        normalize_pair(pa, pt, qn, invt128[:, 0:1])
                        if pend is not None:
                            transpose_to_heads(lambda h: QnT[h], pend[0],
                                               pend[1], "q")
                        pend = (qn, j)
                    transpose_to_heads(lambda h: QnT[h], pend[0], pend[1],
                                       "q")

                    # --- gate (fp16, query slice only, resident) ---
                    for n in range(2):
                        for j, pt in proj_fp16_half(
                                xt16, Wg, bias_t.get("bg"), n, RC, wpa,
                                "wg16h"):
                            nc.scalar.activation(
                                gate16[:, j, n * 512 : (n + 1) * 512],
                                pt[:], AF.Sigmoid)

                # ---- late persistent tiles (group phase) ----
                V8 = pp.tile([128, KC, NH, 66], FP8, tag="v8")
                nc.gpsimd.memset(V8[:, :, :, 64:66], 1.0)
                attnT = pp.tile([128, HP, RS], FP16, tag="attnT")

                # ============ selection / attention bodies ============
                def selection_stages(gi, gp):
                    """returns 4 issue-stage closures for group gi's
                    threshold search (2 count iterations)."""
                    heads = list(range(gi * G, (gi + 1) * G))
                    nt = G * RC
                    st = {}

                    def bracket_update(it):
                        acc, st_t = st["acc"], st["st_t"]
                        st_lo, st_hi = st["st_lo"], st["st_hi"]
                        st_clo, st_chi = st["st_clo"], st["st_chi"]
                        islo = gp.tile([128, nt], U8, tag="islo", bufs=2)
                        nc.vector.tensor_scalar(
                            out=islo[:], in0=acc[:], scalar1=cfg.ATGT,
                            scalar2=None, op0=ALU.is_ge)
                        nc.vector.copy_predicated(st_lo[:], islo[:], st_t[:])
                        nc.vector.copy_predicated(st_clo[:], islo[:], acc[:])
                        ishi = gp.tile([128, nt], U8, tag="ishi", bufs=2)
                        nc.vector.tensor_scalar(
                            out=ishi[:], in0=acc[:], scalar1=cfg.ATGT,
                            scalar2=None, op0=ALU.is_lt)
                        nc.vector.copy_predicated(st_hi[:], ishi[:], st_t[:])
                        nc.vector.copy_predicated(st_chi[:], ishi[:], acc[:])
                        tnew = gp.tile([128, nt], FP32, tag="tnew", bufs=2)
                        if it == 0:
                            nc.vector.tensor_scalar(
                                out=tnew[:], in0=acc[:], scalar1=cfg.ATGT,
                                scalar2=1.0 / cfg.slope0, op0=ALU.subtract,
                                op1=ALU.mult)
                            nc.vector.tensor_add(tnew[:], tnew[:], st_t[:])
                        else:
                            den = gp.tile([128, nt], FP32, tag="den",
                                          bufs=2)
                            nc.vector.tensor_sub(den[:], st_clo[:],
                                                 st_chi[:])
                            nc.vector.tensor_scalar_max(den[:], den[:], 1.0)
                            rden = gp.tile([128, nt], FP32, tag="rden",
                                           bufs=2)
                            nc.vector.reciprocal(rden[:], den[:])
                            nc.vector.tensor_scalar(
                                out=tnew[:], in0=st_clo[:],
                                scalar1=cfg.ATGT,
                                scalar2=None, op0=ALU.subtract)
                            nc.vector.tensor_mul(tnew[:], tnew[:], rden[:])
                            wid = gp.tile([128, nt], FP32, tag="wid",
                                          bufs=2)
                            nc.vector.tensor_sub(wid[:], st_hi[:], st_lo[:])
                            nc.vector.tensor_mul(tnew[:], tnew[:], wid[:])
                            nc.vector.tensor_add(tnew[:], tnew[:], st_lo[:])
                        nc.vector.tensor_tensor(tnew[:], tnew[:], st_lo[:],
                                                ALU.max)
                        nc.vector.tensor_tensor(tnew[:], tnew[:], st_hi[:],
                                                ALU.min)
                        iseq = gp.tile([128, nt], U8, tag="iseq", bufs=2)
                        nc.vector.tensor_scalar(
                            out=iseq[:], in0=acc[:], scalar1=cfg.ATGT,
                            scalar2=None, op0=ALU.not_equal)
                        nc.vector.copy_predicated(st_t[:], iseq[:], tnew[:])

                    def s0():
                        nt0 = gp.tile([128, 1], FP32, tag="nt0")
                        nc.vector.memset(nt0[:], -0.1)
                        for nm, val in (("st_t", 0.1), ("st_lo", -2.1),
                                        ("st_hi", 2.1), ("st_clo", float(SS)),
                                        ("st_chi", float(-SS))):
                            t = gp.tile([128, nt], FP32, tag=nm, name=nm)
                            nc.vector.memset(t[:], val)
                            st[nm] = t
                        st["nt0"] = nt0
                        st["acc"] = gp.tile([128, nt], FP32, tag="acc",
                                            name="acc")
                        s16 = {}
                        for hi_, h in enumerate(heads):
                            for j in range(RC):
                                sp_ = ps512(f"selp_{hi_}_{j}")
                                nc.tensor.matmul(
                                    sp_[:],
                                    QnT[h][0:64, j * 128 : (j + 1) * 128],
                                    KnT[h][0:64, 1 : S : cfg.SUB],
                                    start=True, stop=True)
                                srow = gp.tile([128, SS], FP16,
                                               tag=f"s16_{hi_}_{j}",
                                               name=f"s16_{hi_}_{j}")
                                nc.scalar.activation(srow[:], sp_[:],
                                                     AF.Copy)
                                s16[(hi_, j)] = srow
                        st["s16"] = s16

                    def s1():  # it0 counts on ACT (Sign, acc space)
                        for hi_, h in enumerate(heads):
                            for j in range(RC):
                                col = hi_ * RC + j
                                scr = gp.tile([128, SS], FP8, tag="scr8",
                                              bufs=2, name="scr8")
                                nc.scalar.activation(
                                    scr[:], st["s16"][(hi_, j)][:], AF.Sign,
                                    bias=st["nt0"][:, 0:1],
                                    accum_out=st["acc"][:, col : col + 1])

                    def s2():  # it0 bracket + it1 counts on DVE
                        bracket_update(0)
                        for hi_, h in enumerate(heads):
                            for j in range(RC):
                                col = hi_ * RC + j
                                scr = gp.tile([128, SS], FP8,
                                              tag="scr8", bufs=2,
                                              name="scr8d")
                                nc.vector.tensor_scalar(
                                    out=scr[:], in0=st["s16"][(hi_, j)][:],
                                    scalar1=st["st_t"][:, col : col + 1],
                                    scalar2=None, op0=ALU.is_ge,
                                    op1=ALU.add,
                                    accum_out=st["acc"][:, col : col + 1])
                        nc.vector.tensor_scalar(
                            out=st["acc"][:], in0=st["acc"][:], scalar1=2.0,
                            scalar2=float(-SS), op0=ALU.mult, op1=ALU.add)

                    def s3():  # final bracket + tneg -> QnT rows
                        bracket_update(1)
                        tneg = gp.tile([128, nt], FP16, tag="tneg")
                        nc.vector.tensor_scalar(
                            out=tneg[:], in0=st["st_t"][:], scalar1=-1.0,
                            scalar2=None, op0=ALU.mult)
                        ttp = ps.tile([nt, 128], FP16, tag="tps", bufs=2,
                                      padded_shape=[128, 512], name="ttp")
                        nc.tensor.transpose(ttp[:], tneg[:], ident[:])
                        tnT = gp.tile([nt, 128], FP16, tag="tnT")
                        nc.scalar.activation(tnT[:], ttp[:], AF.Copy)
                        for hi_, h in enumerate(heads):
                            for j in range(RC):
                                col = hi_ * RC + j
                                nc.sync.dma_start(
                                    QnT[h][64:65, j * 128 : (j + 1) * 128],
                                    tnT[col : col + 1, :])

                    return [s0, s1, s2, s3]

                def attention_heads(gi):
                    return [lambda h=h: attention_one(h)
                            for h in range(gi * G, (gi + 1) * G)]

                def attention_one(h):
                    if True:
                        avp = ps.tile([65, RS], FP32, tag="avp", bufs=2,
                                      padded_shape=[128, 512], name="avp")
                        for kcp in range(KC // 2):
                            em8 = pp.tile([128, 2, RS], FP8, tag="em8",
                                          bufs=4, name="em8")
                            stp = ps.tile([128, 2, RS], FP32, tag="pt",
                                          bufs=2,
                                          padded_shape=[128, 2, 512],
                                          name="stp")
                            for sub in range(2):
                                kc = 2 * kcp + sub
                                nc.tensor.matmul(
                                    stp[:, sub, :],
                                    KnT[h][:, kc * 128 : (kc + 1) * 128],
                                    QnT[h][:], start=True, stop=True)
                            e16 = pp.tile([128, 2, RS], FP16, tag="e16",
                                          bufs=2, name="e16")
                            nc.scalar.activation(e16[:], stp[:], AF.Exp)
                            nc.vector.scalar_tensor_tensor(
                                out=em8[:], in0=e16[:],
                                scalar=1.0, in1=e16[:],
                                op0=ALU.is_ge, op1=ALU.mult)
                            nc.tensor.matmul(
                                avp[:],
                                V8[:, 2 * kcp : 2 * kcp + 2, h, 0:65],
                                em8[:, :, :],
                                start=(kcp == 0), stop=(kcp == KC // 2 - 1),
                                perf_mode=DR)
                        # normalize: attnT = avp[0:64] * (1/z); z >= 1 by
                        # construction (the max score always passes t)
                        zrec = pp.tile([1, RS], FP32, tag="zrec", bufs=2,
                                       name="zrec")
                        nc.vector.reciprocal(zrec[:], avp[64:65, :])
                        zrep = pp.tile([64, RS], FP32, tag="zrep", bufs=2,
                                       name="zrep")
                        nc.gpsimd.partition_broadcast(zrep[:], zrec[:])
                        nc.vector.tensor_tensor(
                            attnT[(h % 2) * 64 : (h % 2) * 64 + 64,
                                  h // 2, :],
                            avp[0:64, :], zrep[:], ALU.mult)

                # ===== pipeline: selection(g) stages | attention(g-1) ====
                with (
                    tc.tile_pool(name="poolG0", bufs=1) as gp0,
                    tc.tile_pool(name="poolV", bufs=1) as pv,
                ):
                    stages0 = selection_stages(0, gp0)
                    vgen = proj_fp8(xt8, Wv, bias_t.get("bv"), KC, pv,
                                    "wv8")

                    def vchunks(n):
                        for _ in range(n):
                            j, pt = next(vgen)
                            dst = V8[:, j, :, 0:64]
                            src = pt[:].rearrange("p (h d) -> p h d", h=NH)
                            if j % 2 == 0:
                                nc.scalar.activation(dst, src, AF.Copy)
                            else:
                                nc.vector.tensor_copy(dst, src)

                    for s in stages0:
                        s()
                        vchunks(4)

            # poolX8 closed (xt8 freed)
            for gi in range(1, NG):
                with tc.tile_pool(name=f"poolG{gi}", bufs=1) as gp_:
                    stages = selection_stages(gi, gp_)
                    ah = attention_heads(gi - 1)
                    for s, a in zip(stages, ah):
                        s()
                        a()
            for a in attention_heads(NG - 1):
                a()

            # ================ phase C: out proj + gate ================
            with tc.tile_pool(name="poolC", bufs=1) as pc:
                wo_t = pc.tile([128, HP, D], FP16, tag="wo")
                nc.sync.dma_start(wo_t[:], Wo[:])
                xs_t = pc.tile([128, RC, D], FP32, tag="xs")
                nc.sync.dma_start(xs_t[:], xs[:])
                for j in range(RC):
                    op = pt1024("op_out")
                    for n in range(2):
                        sl = slice(n * 512, (n + 1) * 512)
                        for p in range(HP):
                            nc.tensor.matmul(
                                op[:, sl],
                                attnT[:, p, j * 128 : (j + 1) * 128],
                                wo_t[:, p, sl],
                                start=(p == 0),
                                stop=(p == HP - 1 and not with_bias))
                        if with_bias:
                            nc.tensor.matmul(
                                op[:, sl], ones_h[:], bias_t["bo"][:, sl],
                                start=False, stop=True)
                    dd = pc.tile([128, D], FP32, tag="dd", bufs=2,
                                 name="dd")
                    nc.vector.tensor_sub(dd[:], op[:], xs_t[:, j, :])
                    nc.vector.tensor_mul(dd[:], dd[:], gate16[:, j, :])
                    oo = pc.tile([128, D], FP32, tag="oo", bufs=2,
                                 name="oo")
                    nc.gpsimd.tensor_add(oo[:], dd[:], xs_t[:, j, :])
                    nc.sync.dma_start(out[:, j, :], oo[:])

    nc.finalize()
    return nc


# ---------------------------------------------------------------------------
_NC_CACHE = {}
LAST_EXEC_NS = None
LAST_RESULTS = None


def _get_nc(with_bias: bool):
    key = bool(with_bias)
    if key not in _NC_CACHE:
        _NC_CACHE[key] = build(Cfg(), key)
    return _NC_CACHE[key]


def _pack_core_inputs(x, Wq, bq, Wk, bk, Wv, bv, Wo, bo, Wt, bt, Wg, bg,
                      b, r0, cfg, with_bias, fp8):
    S, D, RS, DCH, HP = cfg.S, cfg.D, cfg.RS, cfg.DCH, cfg.HP
    xb = x[b]
    xt = np.ascontiguousarray(
        np.roll(xb.T, -r0, axis=1).reshape(DCH, 128, S).transpose(1, 0, 2))
    xss = np.ascontiguousarray(
        xb[r0 : r0 + RS].reshape(cfg.RC, 128, D).transpose(1, 0, 2))

    def wpack(W, dt, scale=1.0):
        return np.ascontiguousarray(
            (W * scale).reshape(DCH, 128, D).transpose(1, 0, 2)).astype(dt)

    m = {
        "x8T": xt.astype(fp8),
        "x16T": np.ascontiguousarray(xt[:, :, 0:RS]).astype(np.float16),
        "xs": xss.astype(np.float32),
        "Wq": wpack(Wq, fp8, WSCALE),
        "Wk": wpack(Wk, fp8, WSCALE),
        "Wv": wpack(Wv, fp8, WSCALE),
        "Wg": wpack(Wg, np.float16),
        "Wo": np.ascontiguousarray(
            (Wo / WSCALE).reshape(HP, 128, D).transpose(1, 0, 2))
            .astype(np.float16),
        "Wt": np.ascontiguousarray(
            Wt.reshape(DCH, 128).T * WSCALE).astype(fp8),
        "bt": bt.reshape(1, 1).astype(np.float32),
    }
    if with_bias:
        m["bq"] = (bq * WSCALE).reshape(1, D).astype(np.float16)
        m["bk"] = (bk * WSCALE).reshape(1, D).astype(np.float16)
        m["bv"] = (bv * WSCALE).reshape(1, D).astype(np.float16)
        m["bg"] = bg.reshape(1, D).astype(np.float16)
        m["bo"] = bo.reshape(1, D).astype(np.float16)
    return m


def kernel(**inputs):
    from concourse.bass_utils import run_bass_kernel_spmd
    cfg = Cfg()
    fp8 = mybir.dt.np(FP8)
    x = np.asarray(inputs["x"], np.float32)
    B, S, D = x.shape
    args = [np.asarray(inputs[k]) for k in
            ("Wq", "bq", "Wk", "bk", "Wv", "bv", "Wo", "bo", "Wt", "bt",
             "Wg", "bg")]
    with_bias = any(np.any(np.asarray(inputs[k])) for k in
                    ("bq", "bk", "bv", "bg", "bo"))
    nc = _get_nc(with_bias)
    in_maps = []
    for c in range(8):
        b, q = c // 4, c % 4
        in_maps.append(_pack_core_inputs(
            x, *args, b, q * cfg.RS, cfg, with_bias, fp8))
    trace = bool(int(os.environ.get("KERNEL_TRACE", "0")))
    res = run_bass_kernel_spmd(nc, in_maps, core_ids=list(range(8)),
                               trace=trace)
    global LAST_EXEC_NS, LAST_RESULTS
    LAST_EXEC_NS = res.exec_time_ns
    LAST_RESULTS = res
    out = np.empty((B, S, D), np.float32)
    for c in range(8):
        b, q = c // 4, c % 4
        o = res.results[c]["out"]  # [128, RC, D]
        out[b, q * cfg.RS : (q + 1) * cfg.RS] = \
            o.transpose(1, 0, 2).reshape(cfg.RS, D)
    return out


# revision 58
# speedup vs baseline: 2.3826x; 1.0464x over previous
"""EvolvedAttention Trainium2 Bass kernel (v2).

Full inputs -> full output. Sharding: 8 cores = 2 batches x 4 query-row
slices. Each core computes K/V/attention for its (batch, row-slice) with
all 16 heads; host slices inputs and concatenates row-slice outputs.

v2 design (from ntff trace of v1: DVE 73% busy on top-k counting, PE 38%
and cold):
  - Q/K/V projections in fp8e4 + DoubleRow (weights x32 host-side, folded
    back via Wo/32; cosine normalization cancels the scale for q/k).
  - gate/temp/Wo in fp16.
  - KnT (head-major [65, S], ones row for the threshold trick) and the
    gate stay SBUF-resident; no DRAM staging.
  - top-k threshold found on 4x-subsampled keys (strided matmul rhs),
    3 count-iterations split across ACT (Sign+accum), GPSIMD and DVE,
    bracketed false-position in "acc" space (acc = #ge - #lt).
  - scores recomputed transposed with threshold folded in (K=65), exp on
    ACT PSUM->fp8, mask on GPSIMD (em8 = [z>=0]*e8), AV in fp8 DoubleRow
    with a ones column in V8 for the softmax denominator.
  - selection of group g pipelines against attention of group g-1; the
    V projection fills the group-0 selection bubble.
"""

import os
import numpy as np

import concourse.bass as bass
import concourse.mybir as mybir
import concourse.tile as tile
from concourse import bacc

FP32 = mybir.dt.float32
FP16 = mybir.dt.float16
FP8 = mybir.dt.float8e4
U8 = mybir.dt.uint8
AF = mybir.ActivationFunctionType
ALU = mybir.AluOpType
DR = mybir.MatmulPerfMode.DoubleRow

WSCALE = 32.0


class Cfg:
    def __init__(self):
        self.S = 2048
        self.D = 1024
        self.NH = 16
        self.DH = 64
        self.RS = 512
        self.KK = self.S // 4          # top-k
        self.SUB = 4                   # key subsample for threshold search
        self.SS = self.S // self.SUB   # sampled keys (512)
        self.DCH = self.D // 128       # 8
        self.KC = self.S // 128        # 16
        self.RC = self.RS // 128       # 4
        self.HP = self.NH // 2         # 8
        self.GROUP = 4
        self.NG = self.NH // self.GROUP
        self.n_sel_iters = 2
        # target in acc space: acc = 2*c - SS, c target = KK/SUB
        self.ATGT = float(2 * (self.KK // self.SUB) - self.SS)  # -256
        self.slope0 = 2.0 * 2.8 * self.SS  # d(acc)/dt estimate


def build(cfg: Cfg, with_bias: bool):
    nc = bacc.Bacc()
    S, D, NH, DH, RS = cfg.S, cfg.D, cfg.NH, cfg.DH, cfg.RS
    DCH, KC, RC, HP = cfg.DCH, cfg.KC, cfg.RC, cfg.HP
    SS, G, NG = cfg.SS, cfg.GROUP, cfg.NG

    x8T = nc.dram_tensor("x8T", [128, DCH, S], FP8, kind="ExternalInput")
    x16T = nc.dram_tensor("x16T", [128, DCH, RS], FP16, kind="ExternalInput")
    xs = nc.dram_tensor("xs", [128, RC, D], FP32, kind="ExternalInput")
    Wq = nc.dram_tensor("Wq", [128, DCH, D], FP8, kind="ExternalInput")
    Wk = nc.dram_tensor("Wk", [128, DCH, D], FP8, kind="ExternalInput")
    Wv = nc.dram_tensor("Wv", [128, DCH, D], FP8, kind="ExternalInput")
    Wg = nc.dram_tensor("Wg", [128, DCH, D], FP16, kind="ExternalInput")
    Wo = nc.dram_tensor("Wo", [128, HP, D], FP16, kind="ExternalInput")
    Wt = nc.dram_tensor("Wt", [128, DCH], FP8, kind="ExternalInput")
    bt = nc.dram_tensor("bt", [1, 1], FP32, kind="ExternalInput")
    if with_bias:
        bq = nc.dram_tensor("bq", [1, D], FP16, kind="ExternalInput")
        bk = nc.dram_tensor("bk", [1, D], FP16, kind="ExternalInput")
        bv = nc.dram_tensor("bv", [1, D], FP16, kind="ExternalInput")
        bg = nc.dram_tensor("bg", [1, D], FP16, kind="ExternalInput")
        bo = nc.dram_tensor("bo", [1, D], FP16, kind="ExternalInput")
    out = nc.dram_tensor("out", [128, RC, D], FP32, kind="ExternalOutput")

    with tile.TileContext(nc) as tc:
        with (
            tc.tile_pool(name="persist", bufs=1) as pp,
            tc.tile_pool(name="psum", bufs=2, space="PSUM") as ps,
        ):
            # ---------------- persistent tiles (phase A) ----------------
            ident = pp.tile([128, 128], FP16, tag="ident")
            from concourse.masks import make_identity
            make_identity(nc, ident[:])
            ones_h = pp.tile([1, 128], FP16, tag="ones_h")
            nc.vector.memset(ones_h[:], 1.0)
            KnT = [pp.tile([65, S], FP16, tag=f"knt{h}", name=f"knt{h}")
                   for h in range(NH)]
            QnT = [pp.tile([65, RS], FP16, tag=f"qnt{h}", name=f"qnt{h}")
                   for h in range(NH)]
            for h in range(NH):
                nc.gpsimd.memset(KnT[h][64:65, :], 1.0)
            gate16 = pp.tile([128, RC, D], FP16, tag="gate16")
            invt128 = pp.tile([128, 1], FP32, tag="invt128")
            bt_t = pp.tile([1, 1], FP32, tag="bt")
            nc.sync.dma_start(bt_t[:], bt[:])
            wt_t = pp.tile([128, DCH], FP8, tag="wt")
            nc.sync.dma_start(wt_t[:], Wt[:])
            bias_t = {}
            if with_bias:
                for nm, dram in (("bq", bq), ("bk", bk), ("bv", bv),
                                 ("bg", bg), ("bo", bo)):
                    t = pp.tile([1, D], FP16, tag=nm, name=f"b_{nm}")
                    nc.sync.dma_start(t[:], dram[:])
                    bias_t[nm] = t

            _ptn = [0]

            def pt1024(name):
                """projection psum: [128,1024] (2 banks); rotates over the
                "pt" ring (2 slots) plus the "ptC" slot shared with avp —
                an effective depth-3 ring in phase A."""
                _ptn[0] += 1
                if _ptn[0] % 3 == 0:
                    return ps.tile([128, 1024], FP32, tag="ptC", bufs=1,
                                   padded_shape=[128, 1024], name=name)
                return ps.tile([128, 1024], FP32, tag="pt", bufs=2,
                               padded_shape=[128, 1024], name=name)

            def ps512(name, shape=None, dtype=FP32):
                """small psum ring (transposes, sel-scores, gate, temp)."""
                return ps.tile(shape or [128, 512], dtype, tag="tps",
                               bufs=2, padded_shape=[128, 512], name=name)

            # ---------------- helpers ----------------
            def proj_fp8(xt8, w_dram, bias_row, n_chunks, wpool, wtag):
                """fp8 DoubleRow projection; yields (j, pt) with pt a
                [128,1024] psum row-chunk."""
                w = wpool.tile([128, DCH, D], FP8, tag=wtag, name=wtag,
                               bufs=2)
                nc.sync.dma_start(w[:], w_dram[:])
                for j in range(n_chunks):
                    pt = pt1024(f"pt_{wtag}")
                    for n in range(2):
                        sl = slice(n * 512, (n + 1) * 512)
                        for cp in range(DCH // 2):
                            nc.tensor.matmul(
                                pt[:, sl],
                                xt8[:, 2 * cp : 2 * cp + 2,
                                    j * 128 : (j + 1) * 128],
                                w[:, 2 * cp : 2 * cp + 2, sl],
                                start=(cp == 0),
                                stop=(cp == DCH // 2 - 1 and bias_row is None),
                                perf_mode=DR)
                        if bias_row is not None:
                            nc.tensor.matmul(
                                pt[:, sl], ones_h[:], bias_row[:, sl],
                                start=False, stop=True)
                    yield j, pt

            def proj_fp16_half(xt16, w_dram, bias_row, n, n_chunks, wpool,
                               wtag):
                """one 512-wide output half of an fp16 projection; yields
                (j, pt) psum tiles [128,512]."""
                w = wpool.tile([128, DCH, 512], FP16, tag=wtag, name=wtag,
                               bufs=2)
                nc.sync.dma_start(w[:], w_dram[:, :, n * 512 : (n + 1) * 512])
                for j in range(n_chunks):
                    pt = ps512(f"pt_{wtag}{n}")
                    for c in range(DCH):
                        nc.tensor.matmul(
                            pt[:],
                            xt16[:, c, j * 128 : (j + 1) * 128],
                            w[:, c, :],
                            start=(c == 0),
                            stop=(c == DCH - 1 and bias_row is None))
                    if bias_row is not None:
                        nc.tensor.matmul(
                            pt[:], ones_h[:],
                            bias_row[:, n * 512 : (n + 1) * 512],
                            start=False, stop=True)
                    yield j, pt

            def normalize_pair(sp, pt, dst16, extra_scale_ap):
                """cosine-normalize a [128,1024] psum row-chunk into
                dst16 [128, D] fp16."""
                sq = sp.tile([128, D], FP16, tag="sq", name="sq", bufs=3)
                nc.scalar.activation(sq[:], pt[:], AF.Square)
                n2 = sp.tile([128, NH], FP32, tag="n2", name="n2", bufs=3)
                nc.vector.tensor_reduce(
                    n2[:], sq[:].rearrange("p (h d) -> p h d", h=NH),
                    axis=mybir.AxisListType.X, op=ALU.add)
                rsq = sp.tile([128, NH], FP32, tag="rsq", name="rsq", bufs=3)
                nc.scalar.activation(rsq[:], n2[:], AF.Abs_reciprocal_sqrt)
                if extra_scale_ap is not None:
                    nc.vector.tensor_scalar(
                        out=rsq[:], in0=rsq[:], scalar1=extra_scale_ap,
                        scalar2=None, op0=ALU.mult)
                nc.vector.tensor_tensor(
                    dst16[:].rearrange("p (h d) -> p h d", h=NH),
                    pt[:].rearrange("p (h d) -> p h d", h=NH),
                    rsq[:].rearrange("p (h o) -> p h o", o=1)
                        .to_broadcast([128, NH, DH]),
                    ALU.mult)

            def transpose_to_heads(dst_of_head, src16, j, who):
                """src16 [128 rows, 1024] -> per-head [64, 128] blocks into
                dst_of_head(h)[0:64, j*128:(j+1)*128]."""
                for p in range(HP):
                    tps = ps.tile([128, 128], FP16, tag="tps", bufs=2,
                                  padded_shape=[128, 512], name=f"tps_{who}")
                    nc.tensor.transpose(
                        tps[:], src16[:, p * 128 : (p + 1) * 128], ident[:])
                    for hh in range(2):
                        h = 2 * p + hh
                        dst = dst_of_head(h)[0:64, j * 128 : (j + 1) * 128]
                        src = tps[hh * 64 : hh * 64 + 64, :]
                        if (p + hh + j) % 2 == 0:
                            nc.scalar.activation(dst, src, AF.Copy)
                        else:
                            nc.vector.tensor_copy(dst, src)

            # ================ phase A ================
            with tc.tile_pool(name="poolX8", bufs=1) as px:
                xt8 = px.tile([128, DCH, S], FP8, tag="xt8")
                nc.sync.dma_start(xt8[:], x8T[:])

                with (
                    tc.tile_pool(name="poolA", bufs=1) as pa,
                    tc.tile_pool(name="wpoolA", bufs=2) as wpa,
                ):
                    xt16 = pa.tile([128, DCH, RS], FP16, tag="xt16")
                    nc.sync.dma_start(xt16[:], x16T[:])

                    # --- K projection -> KnT (resident); transposes run
                    # one chunk behind so PE never waits on normalize ---
                    pend = None
                    for j, pt in proj_fp8(xt8, Wk, bias_t.get("bk"),
                                          KC, wpa, "w8"):
                        kn = pa.tile([128, D], FP16, tag="kn", name="kn",
                                     bufs=3)
                        normalize_pair(pa, pt, kn, None)
                        if pend is not None:
                            transpose_to_heads(lambda h: KnT[h], pend[0],
                                               pend[1], "k")
                        pend = (kn, j)
                    transpose_to_heads(lambda h: KnT[h], pend[0], pend[1],
                                       "k")

                    # --- temp (from fp8 x; scale folded into sigmoid) ---
                    tp = ps.tile([1, 512], FP32, tag="tps", bufs=2,
                                 padded_shape=[128, 512], name="tp_temp")
                    first = True
                    for c in range(DCH):
                        for j in range(4):
                            nc.tensor.matmul(
                                tp[:], wt_t[:, c : c + 1],
                                xt8[:, c, j * 512 : (j + 1) * 512],
                                start=first,
                                stop=(c == DCH - 1 and j == 3))
                            first = False
                    tsum = pa.tile([1, 1], FP32, tag="tsum")
                    nc.vector.tensor_reduce(tsum[:], tp[:],
                                            axis=mybir.AxisListType.X,
                                            op=ALU.add)
                    sig = pa.tile([1, 1], FP32, tag="sig")
                    nc.scalar.activation(sig[:], tsum[:], AF.Sigmoid,
                                         bias=bt_t[:],
                                         scale=1.0 / (S * WSCALE))
                    temp = pa.tile([1, 1], FP32, tag="temp")
                    nc.vector.tensor_scalar_add(temp[:], sig[:], 0.5)
                    invt = pa.tile([1, 1], FP32, tag="invt")
                    nc.vector.reciprocal(invt[:], temp[:])
                    nc.gpsimd.partition_broadcast(invt128[:], invt[:])

                    # --- Q projection -> QnT (1/temp folded in) ---
                    pend = None
                    for j, pt in proj_fp8(xt8, Wq, bias_t.get("bq"),
                                          RC, wpa, "w8"):
                        qn = pa.tile([128, D], FP16, tag="qn", name="qn",
                                     bufs=3)
                        normalize_pair(pa, pt, qn, invt128[:, 0:1])
                        if pend is not None:
                            transpose_to_heads(lambda h: QnT[h], pend[0],
                                               pend[1], "q")
                        pend = (qn, j)
                    transpose_to_heads(lambda h: QnT[h], pend[0], pend[1],
                                       "q")

                    # --- gate (fp16, query slice only, resident) ---
                    for n in range(2):
                        for j, pt in proj_fp16_half(
                                xt16, Wg, bias_t.get("bg"), n, RC, wpa,
                                "wg16h"):
                            nc.scalar.activation(
                                gate16[:, j, n * 512 : (n + 1) * 512],
                                pt[:], AF.Sigmoid)

                # ---- late persistent tiles (group phase) ----
                V8 = pp.tile([128, KC, NH, 66], FP8, tag="v8")
                nc.gpsimd.memset(V8[:, :, :, 64:66], 1.0)
                attnT = pp.tile([128, HP, RS], FP16, tag="attnT")

                # ============ selection / attention bodies ============
                def selection_stages(gi, gp):
                    """returns 4 issue-stage closures for group gi's
                    threshold search (2 count iterations)."""
                    heads = list(range(gi * G, (gi + 1) * G))
                    nt = G * RC
                    st = {}

                    def bracket_update(it):
                        acc, st_t = st["acc"], st["st_t"]
                        st_lo, st_hi = st["st_lo"], st["st_hi"]
                        st_clo, st_chi = st["st_clo"], st["st_chi"]
                        islo = gp.tile([128, nt], U8, tag="islo", bufs=2)
                        nc.vector.tensor_scalar(
                            out=islo[:], in0=acc[:], scalar1=cfg.ATGT,
                            scalar2=None, op0=ALU.is_ge)
                        nc.vector.copy_predicated(st_lo[:], islo[:], st_t[:])
                        nc.vector.copy_predicated(st_clo[:], islo[:], acc[:])
                        ishi = gp.tile([128, nt], U8, tag="ishi", bufs=2)
                        nc.vector.tensor_scalar(
                            out=ishi[:], in0=acc[:], scalar1=cfg.ATGT,
                            scalar2=None, op0=ALU.is_lt)
                        nc.vector.copy_predicated(st_hi[:], ishi[:], st_t[:])
                        nc.vector.copy_predicated(st_chi[:], ishi[:], acc[:])
                        tnew = gp.tile([128, nt], FP32, tag="tnew", bufs=2)
                        if it == 0:
                            nc.vector.tensor_scalar(
                                out=tnew[:], in0=acc[:], scalar1=cfg.ATGT,
                                scalar2=1.0 / cfg.slope0, op0=ALU.subtract,
                                op1=ALU.mult)
                            nc.vector.tensor_add(tnew[:], tnew[:], st_t[:])
                        else:
                            den = gp.tile([128, nt], FP32, tag="den",
                                          bufs=2)
                            nc.vector.tensor_sub(den[:], st_clo[:],
                                                 st_chi[:])
                            nc.vector.tensor_scalar_max(den[:], den[:], 1.0)
                            rden = gp.tile([128, nt], FP32, tag="rden",
                                           bufs=2)
                            nc.vector.reciprocal(rden[:], den[:])
                            nc.vector.tensor_scalar(
                                out=tnew[:], in0=st_clo[:],
                                scalar1=cfg.ATGT,
                                scalar2=None, op0=ALU.subtract)
                            nc.vector.tensor_mul(tnew[:], tnew[:], rden[:])
                            wid = gp.tile([128, nt], FP32, tag="wid",
                                          bufs=2)
                            nc.vector.tensor_sub(wid[:], st_hi[:], st_lo[:])
                            nc.vector.tensor_mul(tnew[:], tnew[:], wid[:])
                            nc.vector.tensor_add(tnew[:], tnew[:], st_lo[:])
                        nc.vector.tensor_tensor(tnew[:], tnew[:], st_lo[:],
                                                ALU.max)
                        nc.vector.tensor_tensor(tnew[:], tnew[:], st_hi[:],
                                                ALU.min)
                        iseq = gp.tile([128, nt], U8, tag="iseq", bufs=2)
                        nc.vector.tensor_scalar(
                            out=iseq[:], in0=acc[:], scalar1=cfg.ATGT,
                            scalar2=None, op0=ALU.not_equal)
                        nc.vector.copy_predicated(st_t[:], iseq[:], tnew[:])

                    def s0():
                        nt0 = gp.tile([128, 1], FP32, tag="nt0")
                        nc.vector.memset(nt0[:], -0.1)
                        for nm, val in (("st_t", 0.1), ("st_lo", -2.1),
                                        ("st_hi", 2.1), ("st_clo", float(SS)),
                                        ("st_chi", float(-SS))):
                            t = gp.tile([128, nt], FP32, tag=nm, name=nm)
                            nc.vector.memset(t[:], val)
                            st[nm] = t
                        st["nt0"] = nt0
                        st["acc"] = gp.tile([128, nt], FP32, tag="acc",
                                            name="acc")
                        s16 = {}
                        for hi_, h in enumerate(heads):
                            for j in range(RC):
                                sp_ = ps512(f"selp_{hi_}_{j}")
                                nc.tensor.matmul(
                                    sp_[:],
                                    QnT[h][0:64, j * 128 : (j + 1) * 128],
                                    KnT[h][0:64, 1 : S : cfg.SUB],
                                    start=True, stop=True)
                                srow = gp.tile([128, SS], FP16,
                                               tag=f"s16_{hi_}_{j}",
                                               name=f"s16_{hi_}_{j}")
                                nc.scalar.activation(srow[:], sp_[:],
                                                     AF.Copy)
                                s16[(hi_, j)] = srow
                        st["s16"] = s16

                    def s1():  # it0 counts on ACT (Sign, acc space)
                        for hi_, h in enumerate(heads):
                            for j in range(RC):
                                col = hi_ * RC + j
                                scr = gp.tile([128, SS], FP8, tag="scr8",
                                              bufs=2, name="scr8")
                                nc.scalar.activation(
                                    scr[:], st["s16"][(hi_, j)][:], AF.Sign,
                                    bias=st["nt0"][:, 0:1],
                                    accum_out=st["acc"][:, col : col + 1])

                    def s2():  # it0 bracket + it1 counts on ACT (Sign)
                        bracket_update(0)
                        negt = gp.tile([128, nt], FP32, tag="negt",
                                       name="negt")
                        nc.vector.tensor_scalar(
                            out=negt[:], in0=st["st_t"][:], scalar1=-1.0,
                            scalar2=None, op0=ALU.mult)
                        for hi_, h in enumerate(heads):
                            for j in range(RC):
                                col = hi_ * RC + j
                                scr = gp.tile([128, SS], FP8,
                                              tag="scr8", bufs=2,
                                              name="scr8d")
                                nc.scalar.activation(
                                    scr[:], st["s16"][(hi_, j)][:], AF.Sign,
                                    bias=negt[:, col : col + 1],
                                    accum_out=st["acc"][:, col : col + 1])

                    def s3():  # final bracket + tneg -> QnT rows
                        bracket_update(1)
                        tneg = gp.tile([128, nt], FP16, tag="tneg")
                        nc.vector.tensor_scalar(
                            out=tneg[:], in0=st["st_t"][:], scalar1=-1.0,
                            scalar2=None, op0=ALU.mult)
                        ttp = ps.tile([nt, 128], FP16, tag="tps", bufs=2,
                                      padded_shape=[128, 512], name="ttp")
                        nc.tensor.transpose(ttp[:], tneg[:], ident[:])
                        tnT = gp.tile([nt, 128], FP16, tag="tnT")
                        nc.scalar.activation(tnT[:], ttp[:], AF.Copy)
                        for hi_, h in enumerate(heads):
                            for j in range(RC):
                                col = hi_ * RC + j
                                nc.sync.dma_start(
                                    QnT[h][64:65, j * 128 : (j + 1) * 128],
                                    tnT[col : col + 1, :])

                    return [s0, s1, s2, s3]

                def attention_heads(gi):
                    return [lambda h=h: attention_one(h)
                            for h in range(gi * G, (gi + 1) * G)]

                def attention_one(h):
                    if True:
                        avp = ps.tile([65, RS], FP32, tag="ptC", bufs=1,
                                      padded_shape=[128, 1024], name="avp")
                        for kcp in range(KC // 2):
                            em8 = pp.tile([128, 2, RS], FP8, tag="em8",
                                          bufs=4, name="em8")
                            stp = ps.tile([128, 2, RS], FP32, tag="pt",
                                          bufs=2,
                                          padded_shape=[128, 2, 512],
                                          name="stp")
                            for sub in range(2):
                                kc = 2 * kcp + sub
                                nc.tensor.matmul(
                                    stp[:, sub, :],
                                    KnT[h][:, kc * 128 : (kc + 1) * 128],
                                    QnT[h][:], start=True, stop=True)
                            e16 = pp.tile([128, 2, RS], FP16, tag="e16",
                                          bufs=2, name="e16")
                            nc.scalar.activation(e16[:], stp[:], AF.Exp)
                            nc.vector.scalar_tensor_tensor(
                                out=em8[:], in0=e16[:],
                                scalar=1.0, in1=e16[:],
                                op0=ALU.is_ge, op1=ALU.mult)
                            nc.tensor.matmul(
                                avp[:],
                                V8[:, 2 * kcp : 2 * kcp + 2, h, 0:65],
                                em8[:, :, :],
                                start=(kcp == 0), stop=(kcp == KC // 2 - 1),
                                perf_mode=DR)
                        # normalize: attnT = avp[0:64] * (1/z); z >= 1 by
                        # construction (the max score always passes t)
                        zrec = pp.tile([1, RS], FP32, tag="zrec", bufs=2,
                                       name="zrec")
                        nc.vector.reciprocal(zrec[:], avp[64:65, :])
                        zrep = pp.tile([64, RS], FP32, tag="zrep", bufs=2,
                                       name="zrep")
                        nc.gpsimd.partition_broadcast(zrep[:], zrec[:])
                        nc.vector.tensor_tensor(
                            attnT[(h % 2) * 64 : (h % 2) * 64 + 64,
                                  h // 2, :],
                            avp[0:64, :], zrep[:], ALU.mult)

                # ===== pipeline: selection(g) stages | attention(g-1) ====
                with (
                    tc.tile_pool(name="poolG0", bufs=1) as gp0,
                    tc.tile_pool(name="poolV", bufs=1) as pv,
                ):
                    stages0 = selection_stages(0, gp0)
                    vgen = proj_fp8(xt8, Wv, bias_t.get("bv"), KC, pv,
                                    "wv8")

                    def vchunks(n):
                        for _ in range(n):
                            j, pt = next(vgen)
                            dst = V8[:, j, :, 0:64]
                            src = pt[:].rearrange("p (h d) -> p h d", h=NH)
                            if j % 2 == 0:
                                nc.scalar.activation(dst, src, AF.Copy)
                            else:
                                nc.vector.tensor_copy(dst, src)

                    for s in stages0:
                        s()
                        vchunks(4)

            # poolX8 closed (xt8 freed)
            for gi in range(1, NG):
                with tc.tile_pool(name=f"poolG{gi}", bufs=1) as gp_:
                    stages = selection_stages(gi, gp_)
                    ah = attention_heads(gi - 1)
                    for s, a in zip(stages, ah):
                        s()
                        a()
            for a in attention_heads(NG - 1):
                a()

            # ================ phase C: out proj + gate ================
            with tc.tile_pool(name="poolC", bufs=1) as pc:
                wo_t = pc.tile([128, HP, D], FP16, tag="wo")
                nc.sync.dma_start(wo_t[:], Wo[:])
                xs_t = pc.tile([128, RC, D], FP32, tag="xs")
                nc.sync.dma_start(xs_t[:], xs[:])
                for j in range(RC):
                    op = pt1024("op_out")
                    for n in range(2):
                        sl = slice(n * 512, (n + 1) * 512)
                        for p in range(HP):
                            nc.tensor.matmul(
                                op[:, sl],
                                attnT[:, p, j * 128 : (j + 1) * 128],
                                wo_t[:, p, sl],
                                start=(p == 0),
                                stop=(p == HP - 1 and not with_bias))
                        if with_bias:
                            nc.tensor.matmul(
                                op[:, sl], ones_h[:], bias_t["bo"][:, sl],
                                start=False, stop=True)
                    dd = pc.tile([128, D], FP32, tag="dd", bufs=2,
                                 name="dd")
                    nc.vector.tensor_sub(dd[:], op[:], xs_t[:, j, :])
                    nc.vector.tensor_mul(dd[:], dd[:], gate16[:, j, :])
                    oo = pc.tile([128, D], FP32, tag="oo", bufs=2,
                                 name="oo")
                    nc.gpsimd.tensor_add(oo[:], dd[:], xs_t[:, j, :])
                    nc.sync.dma_start(out[:, j, :], oo[:])

    nc.finalize()
    return nc


# ---------------------------------------------------------------------------
_NC_CACHE = {}
LAST_EXEC_NS = None
LAST_RESULTS = None


def _get_nc(with_bias: bool):
    key = bool(with_bias)
    if key not in _NC_CACHE:
        _NC_CACHE[key] = build(Cfg(), key)
    return _NC_CACHE[key]


def _pack_core_inputs(x, Wq, bq, Wk, bk, Wv, bv, Wo, bo, Wt, bt, Wg, bg,
                      b, r0, cfg, with_bias, fp8):
    S, D, RS, DCH, HP = cfg.S, cfg.D, cfg.RS, cfg.DCH, cfg.HP
    xb = x[b]
    xt = np.ascontiguousarray(
        np.roll(xb.T, -r0, axis=1).reshape(DCH, 128, S).transpose(1, 0, 2))
    xss = np.ascontiguousarray(
        xb[r0 : r0 + RS].reshape(cfg.RC, 128, D).transpose(1, 0, 2))

    def wpack(W, dt, scale=1.0):
        return np.ascontiguousarray(
            (W * scale).reshape(DCH, 128, D).transpose(1, 0, 2)).astype(dt)

    m = {
        "x8T": xt.astype(fp8),
        "x16T": np.ascontiguousarray(xt[:, :, 0:RS]).astype(np.float16),
        "xs": xss.astype(np.float32),
        "Wq": wpack(Wq, fp8, WSCALE),
        "Wk": wpack(Wk, fp8, WSCALE),
        "Wv": wpack(Wv, fp8, WSCALE),
        "Wg": wpack(Wg, np.float16),
        "Wo": np.ascontiguousarray(
            (Wo / WSCALE).reshape(HP, 128, D).transpose(1, 0, 2))
            .astype(np.float16),
        "Wt": np.ascontiguousarray(
            Wt.reshape(DCH, 128).T * WSCALE).astype(fp8),
        "bt": bt.reshape(1, 1).astype(np.float32),
    }
    if with_bias:
        m["bq"] = (bq * WSCALE).reshape(1, D).astype(np.float16)
        m["bk"] = (bk * WSCALE).reshape(1, D).astype(np.float16)
        m["bv"] = (bv * WSCALE).reshape(1, D).astype(np.float16)
        m["bg"] = bg.reshape(1, D).astype(np.float16)
        m["bo"] = bo.reshape(1, D).astype(np.float16)
    return m


def kernel(**inputs):
    from concourse.bass_utils import run_bass_kernel_spmd
    cfg = Cfg()
    fp8 = mybir.dt.np(FP8)
    x = np.asarray(inputs["x"], np.float32)
    B, S, D = x.shape
    args = [np.asarray(inputs[k]) for k in
            ("Wq", "bq", "Wk", "bk", "Wv", "bv", "Wo", "bo", "Wt", "bt",
             "Wg", "bg")]
    with_bias = any(np.any(np.asarray(inputs[k])) for k in
                    ("bq", "bk", "bv", "bg", "bo"))
    nc = _get_nc(with_bias)
    in_maps = []
    for c in range(8):
        b, q = c // 4, c % 4
        in_maps.append(_pack_core_inputs(
            x, *args, b, q * cfg.RS, cfg, with_bias, fp8))
    trace = bool(int(os.environ.get("KERNEL_TRACE", "0")))
    res = run_bass_kernel_spmd(nc, in_maps, core_ids=list(range(8)),
                               trace=trace)
    global LAST_EXEC_NS, LAST_RESULTS
    LAST_EXEC_NS = res.exec_time_ns
    LAST_RESULTS = res
    out = np.empty((B, S, D), np.float32)
    for c in range(8):
        b, q = c // 4, c % 4
        o = res.results[c]["out"]  # [128, RC, D]
        out[b, q * cfg.RS : (q + 1) * cfg.RS] = \
            o.transpose(1, 0, 2).reshape(cfg.RS, D)
    return out


# revision 59
# speedup vs baseline: 2.4088x; 1.0110x over previous
"""EvolvedAttention Trainium2 Bass kernel (v2).

Full inputs -> full output. Sharding: 8 cores = 2 batches x 4 query-row
slices. Each core computes K/V/attention for its (batch, row-slice) with
all 16 heads; host slices inputs and concatenates row-slice outputs.

v2 design (from ntff trace of v1: DVE 73% busy on top-k counting, PE 38%
and cold):
  - Q/K/V projections in fp8e4 + DoubleRow (weights x32 host-side, folded
    back via Wo/32; cosine normalization cancels the scale for q/k).
  - gate/temp/Wo in fp16.
  - KnT (head-major [65, S], ones row for the threshold trick) and the
    gate stay SBUF-resident; no DRAM staging.
  - top-k threshold found on 4x-subsampled keys (strided matmul rhs),
    3 count-iterations split across ACT (Sign+accum), GPSIMD and DVE,
    bracketed false-position in "acc" space (acc = #ge - #lt).
  - scores recomputed transposed with threshold folded in (K=65), exp on
    ACT PSUM->fp8, mask on GPSIMD (em8 = [z>=0]*e8), AV in fp8 DoubleRow
    with a ones column in V8 for the softmax denominator.
  - selection of group g pipelines against attention of group g-1; the
    V projection fills the group-0 selection bubble.
"""

import os
import numpy as np

import concourse.bass as bass
import concourse.mybir as mybir
import concourse.tile as tile
from concourse import bacc

FP32 = mybir.dt.float32
FP16 = mybir.dt.float16
FP8 = mybir.dt.float8e4
U8 = mybir.dt.uint8
AF = mybir.ActivationFunctionType
ALU = mybir.AluOpType
DR = mybir.MatmulPerfMode.DoubleRow

WSCALE = 32.0


class Cfg:
    def __init__(self):
        self.S = 2048
        self.D = 1024
        self.NH = 16
        self.DH = 64
        self.RS = 512
        self.KK = self.S // 4          # top-k
        self.SUB = 4                   # key subsample for threshold search
        self.SS = self.S // self.SUB   # sampled keys (512)
        self.DCH = self.D // 128       # 8
        self.KC = self.S // 128        # 16
        self.RC = self.RS // 128       # 4
        self.HP = self.NH // 2         # 8
        self.GROUP = 4
        self.NG = self.NH // self.GROUP
        self.n_sel_iters = 2
        # target in acc space: acc = 2*c - SS, c target = KK/SUB
        self.ATGT = float(2 * (self.KK // self.SUB) - self.SS)  # -256
        self.slope0 = 2.0 * 2.8 * self.SS  # d(acc)/dt estimate


def build(cfg: Cfg, with_bias: bool):
    nc = bacc.Bacc()
    S, D, NH, DH, RS = cfg.S, cfg.D, cfg.NH, cfg.DH, cfg.RS
    DCH, KC, RC, HP = cfg.DCH, cfg.KC, cfg.RC, cfg.HP
    SS, G, NG = cfg.SS, cfg.GROUP, cfg.NG

    x8T = nc.dram_tensor("x8T", [128, DCH, S], FP8, kind="ExternalInput")
    x16T = nc.dram_tensor("x16T", [128, DCH, RS], FP16, kind="ExternalInput")
    xs = nc.dram_tensor("xs", [128, RC, D], FP32, kind="ExternalInput")
    Wq = nc.dram_tensor("Wq", [128, DCH, D], FP8, kind="ExternalInput")
    Wk = nc.dram_tensor("Wk", [128, DCH, D], FP8, kind="ExternalInput")
    Wv = nc.dram_tensor("Wv", [128, DCH, D], FP8, kind="ExternalInput")
    Wg = nc.dram_tensor("Wg", [128, DCH, D], FP16, kind="ExternalInput")
    Wo = nc.dram_tensor("Wo", [128, HP, D], FP16, kind="ExternalInput")
    Wt = nc.dram_tensor("Wt", [128, DCH], FP8, kind="ExternalInput")
    bt = nc.dram_tensor("bt", [1, 1], FP32, kind="ExternalInput")
    if with_bias:
        bq = nc.dram_tensor("bq", [1, D], FP16, kind="ExternalInput")
        bk = nc.dram_tensor("bk", [1, D], FP16, kind="ExternalInput")
        bv = nc.dram_tensor("bv", [1, D], FP16, kind="ExternalInput")
        bg = nc.dram_tensor("bg", [1, D], FP16, kind="ExternalInput")
        bo = nc.dram_tensor("bo", [1, D], FP16, kind="ExternalInput")
    out = nc.dram_tensor("out", [128, RC, D], FP32, kind="ExternalOutput")

    with tile.TileContext(nc) as tc:
        with (
            tc.tile_pool(name="persist", bufs=1) as pp,
            tc.tile_pool(name="psum", bufs=2, space="PSUM") as ps,
        ):
            # ---------------- persistent tiles (phase A) ----------------
            ident = pp.tile([128, 128], FP16, tag="ident")
            from concourse.masks import make_identity
            make_identity(nc, ident[:])
            ones_h = pp.tile([1, 128], FP16, tag="ones_h")
            nc.vector.memset(ones_h[:], 1.0)
            KnT = [pp.tile([65, S], FP16, tag=f"knt{h}", name=f"knt{h}")
                   for h in range(NH)]
            QnT = [pp.tile([65, RS], FP16, tag=f"qnt{h}", name=f"qnt{h}")
                   for h in range(NH)]
            for h in range(NH):
                nc.gpsimd.memset(KnT[h][64:65, :], 1.0)
            gate16 = pp.tile([128, RC, D], FP16, tag="gate16")
            invt128 = pp.tile([128, 1], FP32, tag="invt128")
            bt_t = pp.tile([1, 1], FP32, tag="bt")
            nc.sync.dma_start(bt_t[:], bt[:])
            wt_t = pp.tile([128, DCH], FP8, tag="wt")
            nc.sync.dma_start(wt_t[:], Wt[:])
            bias_t = {}
            if with_bias:
                for nm, dram in (("bq", bq), ("bk", bk), ("bv", bv),
                                 ("bg", bg), ("bo", bo)):
                    t = pp.tile([1, D], FP16, tag=nm, name=f"b_{nm}")
                    nc.sync.dma_start(t[:], dram[:])
                    bias_t[nm] = t

            _ptn = [0]

            def pt1024(name):
                """projection psum: [128,1024] (2 banks); rotates over the
                "pt" ring (2 slots) plus the "ptC" slot shared with avp —
                an effective depth-3 ring in phase A."""
                _ptn[0] += 1
                if _ptn[0] % 3 == 0:
                    return ps.tile([128, 1024], FP32, tag="ptC", bufs=1,
                                   padded_shape=[128, 1024], name=name)
                return ps.tile([128, 1024], FP32, tag="pt", bufs=2,
                               padded_shape=[128, 1024], name=name)

            def ps512(name, shape=None, dtype=FP32):
                """small psum ring (transposes, sel-scores, gate, temp)."""
                return ps.tile(shape or [128, 512], dtype, tag="tps",
                               bufs=2, padded_shape=[128, 512], name=name)

            # ---------------- helpers ----------------
            def proj_fp8(xt8, w_dram, bias_row, n_chunks, wpool, wtag):
                """fp8 DoubleRow projection; yields (j, pt) with pt a
                [128,1024] psum row-chunk."""
                w = wpool.tile([128, DCH, D], FP8, tag=wtag, name=wtag,
                               bufs=2)
                nc.sync.dma_start(w[:], w_dram[:])
                for j in range(n_chunks):
                    pt = pt1024(f"pt_{wtag}")
                    for n in range(2):
                        sl = slice(n * 512, (n + 1) * 512)
                        for cp in range(DCH // 2):
                            nc.tensor.matmul(
                                pt[:, sl],
                                xt8[:, 2 * cp : 2 * cp + 2,
                                    j * 128 : (j + 1) * 128],
                                w[:, 2 * cp : 2 * cp + 2, sl],
                                start=(cp == 0),
                                stop=(cp == DCH // 2 - 1 and bias_row is None),
                                perf_mode=DR)
                        if bias_row is not None:
                            nc.tensor.matmul(
                                pt[:, sl], ones_h[:], bias_row[:, sl],
                                start=False, stop=True)
                    yield j, pt

            def proj_fp16_half(xt16, w_dram, bias_row, n, n_chunks, wpool,
                               wtag):
                """one 512-wide output half of an fp16 projection; yields
                (j, pt) psum tiles [128,512]."""
                w = wpool.tile([128, DCH, 512], FP16, tag=wtag, name=wtag,
                               bufs=2)
                nc.sync.dma_start(w[:], w_dram[:, :, n * 512 : (n + 1) * 512])
                for j in range(n_chunks):
                    pt = ps512(f"pt_{wtag}{n}")
                    for c in range(DCH):
                        nc.tensor.matmul(
                            pt[:],
                            xt16[:, c, j * 128 : (j + 1) * 128],
                            w[:, c, :],
                            start=(c == 0),
                            stop=(c == DCH - 1 and bias_row is None))
                    if bias_row is not None:
                        nc.tensor.matmul(
                            pt[:], ones_h[:],
                            bias_row[:, n * 512 : (n + 1) * 512],
                            start=False, stop=True)
                    yield j, pt

            def normalize_pair(sp, pt, dst16, extra_scale_ap):
                """cosine-normalize a [128,1024] psum row-chunk into
                dst16 [128, D] fp16."""
                sq = sp.tile([128, D], FP16, tag="sq", name="sq", bufs=3)
                nc.scalar.activation(sq[:], pt[:], AF.Square)
                n2 = sp.tile([128, NH], FP32, tag="n2", name="n2", bufs=3)
                nc.vector.tensor_reduce(
                    n2[:], sq[:].rearrange("p (h d) -> p h d", h=NH),
                    axis=mybir.AxisListType.X, op=ALU.add)
                rsq = sp.tile([128, NH], FP32, tag="rsq", name="rsq", bufs=3)
                nc.scalar.activation(rsq[:], n2[:], AF.Abs_reciprocal_sqrt)
                if extra_scale_ap is not None:
                    nc.vector.tensor_scalar(
                        out=rsq[:], in0=rsq[:], scalar1=extra_scale_ap,
                        scalar2=None, op0=ALU.mult)
                nc.vector.tensor_tensor(
                    dst16[:].rearrange("p (h d) -> p h d", h=NH),
                    pt[:].rearrange("p (h d) -> p h d", h=NH),
                    rsq[:].rearrange("p (h o) -> p h o", o=1)
                        .to_broadcast([128, NH, DH]),
                    ALU.mult)

            def transpose_to_heads(dst_of_head, src16, j, who):
                """src16 [128 rows, 1024] -> per-head [64, 128] blocks into
                dst_of_head(h)[0:64, j*128:(j+1)*128]."""
                for p in range(HP):
                    tps = ps.tile([128, 128], FP16, tag="tps", bufs=2,
                                  padded_shape=[128, 512], name=f"tps_{who}")
                    nc.tensor.transpose(
                        tps[:], src16[:, p * 128 : (p + 1) * 128], ident[:])
                    for hh in range(2):
                        h = 2 * p + hh
                        dst = dst_of_head(h)[0:64, j * 128 : (j + 1) * 128]
                        src = tps[hh * 64 : hh * 64 + 64, :]
                        if (p + hh + j) % 2 == 0:
                            nc.scalar.activation(dst, src, AF.Copy)
                        else:
                            nc.vector.tensor_copy(dst, src)

            # ================ phase A ================
            with tc.tile_pool(name="poolX8", bufs=1) as px:
                xt8 = px.tile([128, DCH, S], FP8, tag="xt8")
                nc.sync.dma_start(xt8[:], x8T[:])

                with (
                    tc.tile_pool(name="poolA", bufs=1) as pa,
                    tc.tile_pool(name="wpoolA", bufs=2) as wpa,
                ):
                    xt16 = pa.tile([128, DCH, RS], FP16, tag="xt16")
                    nc.sync.dma_start(xt16[:], x16T[:])

                    # --- K projection -> KnT (resident); transposes run
                    # one chunk behind so PE never waits on normalize ---
                    pend = None
                    for j, pt in proj_fp8(xt8, Wk, bias_t.get("bk"),
                                          KC, wpa, "w8"):
                        kn = pa.tile([128, D], FP16, tag="kn", name="kn",
                                     bufs=3)
                        normalize_pair(pa, pt, kn, None)
                        if pend is not None:
                            transpose_to_heads(lambda h: KnT[h], pend[0],
                                               pend[1], "k")
                        pend = (kn, j)
                    transpose_to_heads(lambda h: KnT[h], pend[0], pend[1],
                                       "k")

                    # --- temp (from fp8 x; scale folded into sigmoid) ---
                    tp = ps.tile([1, 512], FP32, tag="tps", bufs=2,
                                 padded_shape=[128, 512], name="tp_temp")
                    first = True
                    for c in range(DCH):
                        for j in range(4):
                            nc.tensor.matmul(
                                tp[:], wt_t[:, c : c + 1],
                                xt8[:, c, j * 512 : (j + 1) * 512],
                                start=first,
                                stop=(c == DCH - 1 and j == 3))
                            first = False
                    tsum = pa.tile([1, 1], FP32, tag="tsum")
                    nc.vector.tensor_reduce(tsum[:], tp[:],
                                            axis=mybir.AxisListType.X,
                                            op=ALU.add)
                    sig = pa.tile([1, 1], FP32, tag="sig")
                    nc.scalar.activation(sig[:], tsum[:], AF.Sigmoid,
                                         bias=bt_t[:],
                                         scale=1.0 / (S * WSCALE))
                    temp = pa.tile([1, 1], FP32, tag="temp")
                    nc.vector.tensor_scalar_add(temp[:], sig[:], 0.5)
                    invt = pa.tile([1, 1], FP32, tag="invt")
                    nc.vector.reciprocal(invt[:], temp[:])
                    nc.gpsimd.partition_broadcast(invt128[:], invt[:])

                    # --- Q projection -> QnT (1/temp folded in) ---
                    pend = None
                    for j, pt in proj_fp8(xt8, Wq, bias_t.get("bq"),
                                          RC, wpa, "w8"):
                        qn = pa.tile([128, D], FP16, tag="qn", name="qn",
                                     bufs=3)
                        normalize_pair(pa, pt, qn, invt128[:, 0:1])
                        if pend is not None:
                            transpose_to_heads(lambda h: QnT[h], pend[0],
                                               pend[1], "q")
                        pend = (qn, j)
                    transpose_to_heads(lambda h: QnT[h], pend[0], pend[1],
                                       "q")

                    # --- gate (fp16, query slice only, resident) ---
                    for n in range(2):
                        for j, pt in proj_fp16_half(
                                xt16, Wg, bias_t.get("bg"), n, RC, wpa,
                                "wg16h"):
                            nc.scalar.activation(
                                gate16[:, j, n * 512 : (n + 1) * 512],
                                pt[:], AF.Sigmoid)

                # ---- late persistent tiles (group phase) ----
                V8 = pp.tile([128, KC, NH, 66], FP8, tag="v8")
                nc.gpsimd.memset(V8[:, :, :, 64:66], 1.0)
                attnT = pp.tile([128, HP, RS], FP16, tag="attnT")

                # ============ selection / attention bodies ============
                def selection_stages(gi, gp):
                    """returns 4 issue-stage closures for group gi's
                    threshold search (2 count iterations)."""
                    heads = list(range(gi * G, (gi + 1) * G))
                    nt = G * RC
                    st = {}

                    def bracket_update(it):
                        acc, st_t = st["acc"], st["st_t"]
                        st_lo, st_hi = st["st_lo"], st["st_hi"]
                        st_clo, st_chi = st["st_clo"], st["st_chi"]
                        islo = gp.tile([128, nt], U8, tag="islo", bufs=2)
                        nc.vector.tensor_scalar(
                            out=islo[:], in0=acc[:], scalar1=cfg.ATGT,
                            scalar2=None, op0=ALU.is_ge)
                        nc.vector.copy_predicated(st_lo[:], islo[:], st_t[:])
                        nc.vector.copy_predicated(st_clo[:], islo[:], acc[:])
                        ishi = gp.tile([128, nt], U8, tag="ishi", bufs=2)
                        nc.vector.tensor_scalar(
                            out=ishi[:], in0=acc[:], scalar1=cfg.ATGT,
                            scalar2=None, op0=ALU.is_lt)
                        nc.vector.copy_predicated(st_hi[:], ishi[:], st_t[:])
                        nc.vector.copy_predicated(st_chi[:], ishi[:], acc[:])
                        tnew = gp.tile([128, nt], FP32, tag="tnew", bufs=2)
                        if it == 0:
                            nc.vector.tensor_scalar(
                                out=tnew[:], in0=acc[:], scalar1=cfg.ATGT,
                                scalar2=1.0 / cfg.slope0, op0=ALU.subtract,
                                op1=ALU.mult)
                            nc.vector.tensor_add(tnew[:], tnew[:], st_t[:])
                        else:
                            den = gp.tile([128, nt], FP32, tag="den",
                                          bufs=2)
                            nc.vector.tensor_sub(den[:], st_clo[:],
                                                 st_chi[:])
                            nc.vector.tensor_scalar_max(den[:], den[:], 1.0)
                            rden = gp.tile([128, nt], FP32, tag="rden",
                                           bufs=2)
                            nc.vector.reciprocal(rden[:], den[:])
                            nc.vector.tensor_scalar(
                                out=tnew[:], in0=st_clo[:],
                                scalar1=cfg.ATGT,
                                scalar2=None, op0=ALU.subtract)
                            nc.vector.tensor_mul(tnew[:], tnew[:], rden[:])
                            wid = gp.tile([128, nt], FP32, tag="wid",
                                          bufs=2)
                            nc.vector.tensor_sub(wid[:], st_hi[:], st_lo[:])
                            nc.vector.tensor_mul(tnew[:], tnew[:], wid[:])
                            nc.vector.tensor_add(tnew[:], tnew[:], st_lo[:])
                        nc.vector.tensor_tensor(tnew[:], tnew[:], st_lo[:],
                                                ALU.max)
                        nc.vector.tensor_tensor(tnew[:], tnew[:], st_hi[:],
                                                ALU.min)
                        iseq = gp.tile([128, nt], U8, tag="iseq", bufs=2)
                        nc.vector.tensor_scalar(
                            out=iseq[:], in0=acc[:], scalar1=cfg.ATGT,
                            scalar2=None, op0=ALU.not_equal)
                        nc.vector.copy_predicated(st_t[:], iseq[:], tnew[:])

                    def s0():
                        nt0 = gp.tile([128, 1], FP32, tag="nt0")
                        nc.vector.memset(nt0[:], -0.1)
                        for nm, val in (("st_t", 0.1), ("st_lo", -2.1),
                                        ("st_hi", 2.1), ("st_clo", float(SS)),
                                        ("st_chi", float(-SS))):
                            t = gp.tile([128, nt], FP32, tag=nm, name=nm)
                            nc.vector.memset(t[:], val)
                            st[nm] = t
                        st["nt0"] = nt0
                        st["acc"] = gp.tile([128, nt], FP32, tag="acc",
                                            name="acc")
                        s16 = {}
                        for hi_, h in enumerate(heads):
                            for j in range(RC):
                                sp_ = ps512(f"selp_{hi_}_{j}")
                                nc.tensor.matmul(
                                    sp_[:],
                                    QnT[h][0:64, j * 128 : (j + 1) * 128],
                                    KnT[h][0:64, 1 : S : cfg.SUB],
                                    start=True, stop=True)
                                srow = gp.tile([128, SS], FP16,
                                               tag=f"s16_{hi_}_{j}",
                                               name=f"s16_{hi_}_{j}")
                                nc.scalar.activation(srow[:], sp_[:],
                                                     AF.Copy)
                                s16[(hi_, j)] = srow
                        st["s16"] = s16

                    def s1():  # it0 counts on ACT (Sign, acc space)
                        for hi_, h in enumerate(heads):
                            for j in range(RC):
                                col = hi_ * RC + j
                                scr = gp.tile([128, SS], FP8, tag="scr8",
                                              bufs=2, name="scr8")
                                nc.scalar.activation(
                                    scr[:], st["s16"][(hi_, j)][:], AF.Sign,
                                    bias=st["nt0"][:, 0:1],
                                    accum_out=st["acc"][:, col : col + 1])

                    def s2():  # it0 bracket + it1 counts on ACT (Sign)
                        bracket_update(0)
                        negt = gp.tile([128, nt], FP32, tag="negt",
                                       name="negt")
                        nc.vector.tensor_scalar(
                            out=negt[:], in0=st["st_t"][:], scalar1=-1.0,
                            scalar2=None, op0=ALU.mult)
                        for hi_, h in enumerate(heads):
                            for j in range(RC):
                                col = hi_ * RC + j
                                scr = gp.tile([128, SS], FP8,
                                              tag="scr8", bufs=2,
                                              name="scr8d")
                                nc.scalar.activation(
                                    scr[:], st["s16"][(hi_, j)][:], AF.Sign,
                                    bias=negt[:, col : col + 1],
                                    accum_out=st["acc"][:, col : col + 1])

                    def s3():  # final bracket + tneg -> QnT rows
                        bracket_update(1)
                        tneg = gp.tile([128, nt], FP16, tag="tneg")
                        nc.vector.tensor_scalar(
                            out=tneg[:], in0=st["st_t"][:], scalar1=-1.0,
                            scalar2=None, op0=ALU.mult)
                        ttp = ps.tile([nt, 128], FP16, tag="tps", bufs=2,
                                      padded_shape=[128, 512], name="ttp")
                        nc.tensor.transpose(ttp[:], tneg[:], ident[:])
                        tnT = gp.tile([nt, 128], FP16, tag="tnT")
                        nc.scalar.activation(tnT[:], ttp[:], AF.Copy)
                        for hi_, h in enumerate(heads):
                            # one DMA per head: rows hi_*RC..+RC of tnT are
                            # the RC j-slices of QnT[h]'s threshold row
                            nc.sync.dma_start(
                                QnT[h][64:65, :],
                                tnT[hi_ * RC : (hi_ + 1) * RC, :])

                    return [s0, s1, s2, s3]

                def attention_heads(gi):
                    return [lambda h=h: attention_one(h)
                            for h in range(gi * G, (gi + 1) * G)]

                def attention_one(h):
                    if True:
                        avp = ps.tile([65, RS], FP32, tag="ptC", bufs=1,
                                      padded_shape=[128, 1024], name="avp")
                        for kcp in range(KC // 2):
                            em8 = pp.tile([128, 2, RS], FP8, tag="em8",
                                          bufs=4, name="em8")
                            stp = ps.tile([128, 2, RS], FP32, tag="pt",
                                          bufs=2,
                                          padded_shape=[128, 2, 512],
                                          name="stp")
                            for sub in range(2):
                                kc = 2 * kcp + sub
                                nc.tensor.matmul(
                                    stp[:, sub, :],
                                    KnT[h][:, kc * 128 : (kc + 1) * 128],
                                    QnT[h][:], start=True, stop=True)
                            e16 = pp.tile([128, 2, RS], FP16, tag="e16",
                                          bufs=2, name="e16")
                            nc.scalar.activation(e16[:], stp[:], AF.Exp)
                            nc.vector.scalar_tensor_tensor(
                                out=em8[:], in0=e16[:],
                                scalar=1.0, in1=e16[:],
                                op0=ALU.is_ge, op1=ALU.mult)
                            nc.tensor.matmul(
                                avp[:],
                                V8[:, 2 * kcp : 2 * kcp + 2, h, 0:65],
                                em8[:, :, :],
                                start=(kcp == 0), stop=(kcp == KC // 2 - 1),
                                perf_mode=DR)
                        # normalize: attnT = avp[0:64] * (1/z); z >= 1 by
                        # construction (the max score always passes t)
                        zrec = pp.tile([1, RS], FP32, tag="zrec", bufs=2,
                                       name="zrec")
                        nc.vector.reciprocal(zrec[:], avp[64:65, :])
                        zrep = pp.tile([64, RS], FP32, tag="zrep", bufs=2,
                                       name="zrep")
                        nc.gpsimd.partition_broadcast(zrep[:], zrec[:])
                        nc.vector.tensor_tensor(
                            attnT[(h % 2) * 64 : (h % 2) * 64 + 64,
                                  h // 2, :],
                            avp[0:64, :], zrep[:], ALU.mult)

                # ===== pipeline: selection(g) stages | attention(g-1) ====
                with (
                    tc.tile_pool(name="poolG0", bufs=1) as gp0,
                    tc.tile_pool(name="poolV", bufs=1) as pv,
                ):
                    stages0 = selection_stages(0, gp0)
                    vgen = proj_fp8(xt8, Wv, bias_t.get("bv"), KC, pv,
                                    "wv8")

                    def vchunks(n):
                        for _ in range(n):
                            j, pt = next(vgen)
                            dst = V8[:, j, :, 0:64]
                            src = pt[:].rearrange("p (h d) -> p h d", h=NH)
                            if j % 2 == 0:
                                nc.scalar.activation(dst, src, AF.Copy)
                            else:
                                nc.vector.tensor_copy(dst, src)

                    for s in stages0:
                        s()
                        vchunks(4)

            # poolX8 closed (xt8 freed)
            for gi in range(1, NG):
                with tc.tile_pool(name=f"poolG{gi}", bufs=1) as gp_:
                    stages = selection_stages(gi, gp_)
                    ah = attention_heads(gi - 1)
                    for s, a in zip(stages, ah):
                        s()
                        a()
            for a in attention_heads(NG - 1):
                a()

            # ================ phase C: out proj + gate ================
            with tc.tile_pool(name="poolC", bufs=1) as pc:
                wo_t = pc.tile([128, HP, D], FP16, tag="wo")
                nc.sync.dma_start(wo_t[:], Wo[:])
                xs_t = pc.tile([128, RC, D], FP32, tag="xs")
                nc.sync.dma_start(xs_t[:], xs[:])
                for j in range(RC):
                    op = pt1024("op_out")
                    for n in range(2):
                        sl = slice(n * 512, (n + 1) * 512)
                        for p in range(HP):
                            nc.tensor.matmul(
                                op[:, sl],
                                attnT[:, p, j * 128 : (j + 1) * 128],
                                wo_t[:, p, sl],
                                start=(p == 0),
                                stop=(p == HP - 1 and not with_bias))
                        if with_bias:
                            nc.tensor.matmul(
                                op[:, sl], ones_h[:], bias_t["bo"][:, sl],
                                start=False, stop=True)
                    dd = pc.tile([128, D], FP32, tag="dd", bufs=2,
                                 name="dd")
                    nc.vector.tensor_sub(dd[:], op[:], xs_t[:, j, :])
                    nc.vector.tensor_mul(dd[:], dd[:], gate16[:, j, :])
                    oo = pc.tile([128, D], FP32, tag="oo", bufs=2,
                                 name="oo")
                    nc.gpsimd.tensor_add(oo[:], dd[:], xs_t[:, j, :])
                    nc.sync.dma_start(out[:, j, :], oo[:])

    nc.finalize()
    return nc


# ---------------------------------------------------------------------------
_NC_CACHE = {}
LAST_EXEC_NS = None
LAST_RESULTS = None


def _get_nc(with_bias: bool):
    key = bool(with_bias)
    if key not in _NC_CACHE:
        _NC_CACHE[key] = build(Cfg(), key)
    return _NC_CACHE[key]


def _pack_core_inputs(x, Wq, bq, Wk, bk, Wv, bv, Wo, bo, Wt, bt, Wg, bg,
                      b, r0, cfg, with_bias, fp8):
    S, D, RS, DCH, HP = cfg.S, cfg.D, cfg.RS, cfg.DCH, cfg.HP
    xb = x[b]
    xt = np.ascontiguousarray(
        np.roll(xb.T, -r0, axis=1).reshape(DCH, 128, S).transpose(1, 0, 2))
    xss = np.ascontiguousarray(
        xb[r0 : r0 + RS].reshape(cfg.RC, 128, D).transpose(1, 0, 2))

    def wpack(W, dt, scale=1.0):
        return np.ascontiguousarray(
            (W * scale).reshape(DCH, 128, D).transpose(1, 0, 2)).astype(dt)

    m = {
        "x8T": xt.astype(fp8),
        "x16T": np.ascontiguousarray(xt[:, :, 0:RS]).astype(np.float16),
        "xs": xss.astype(np.float32),
        "Wq": wpack(Wq, fp8, WSCALE),
        "Wk": wpack(Wk, fp8, WSCALE),
        "Wv": wpack(Wv, fp8, WSCALE),
        "Wg": wpack(Wg, np.float16),
        "Wo": np.ascontiguousarray(
            (Wo / WSCALE).reshape(HP, 128, D).transpose(1, 0, 2))
            .astype(np.float16),
        "Wt": np.ascontiguousarray(
            Wt.reshape(DCH, 128).T * WSCALE).astype(fp8),
        "bt": bt.reshape(1, 1).astype(np.float32),
    }
    if with_bias:
        m["bq"] = (bq * WSCALE).reshape(1, D).astype(np.float16)
        m["bk"] = (bk * WSCALE).reshape(1, D).astype(np.float16)
        m["bv"] = (bv * WSCALE).reshape(1, D).astype(np.float16)
        m["bg"] = bg.reshape(1, D).astype(np.float16)
        m["bo"] = bo.reshape(1, D).astype(np.float16)
    return m


def kernel(**inputs):
    from concourse.bass_utils import run_bass_kernel_spmd
    cfg = Cfg()
    fp8 = mybir.dt.np(FP8)
    x = np.asarray(inputs["x"], np.float32)
    B, S, D = x.shape
    args = [np.asarray(inputs[k]) for k in
            ("Wq", "bq", "Wk", "bk", "Wv", "bv", "Wo", "bo", "Wt", "bt",
             "Wg", "bg")]
    with_bias = any(np.any(np.asarray(inputs[k])) for k in
                    ("bq", "bk", "bv", "bg", "bo"))
    nc = _get_nc(with_bias)
    in_maps = []
    for c in range(8):
        b, q = c // 4, c % 4
        in_maps.append(_pack_core_inputs(
            x, *args, b, q * cfg.RS, cfg, with_bias, fp8))
    trace = bool(int(os.environ.get("KERNEL_TRACE", "0")))
    res = run_bass_kernel_spmd(nc, in_maps, core_ids=list(range(8)),
                               trace=trace)
    global LAST_EXEC_NS, LAST_RESULTS
    LAST_EXEC_NS = res.exec_time_ns
    LAST_RESULTS = res
    out = np.empty((B, S, D), np.float32)
    for c in range(8):
        b, q = c // 4, c % 4
        o = res.results[c]["out"]  # [128, RC, D]
        out[b, q * cfg.RS : (q + 1) * cfg.RS] = \
            o.transpose(1, 0, 2).reshape(cfg.RS, D)
    return out
